# revision 1
# baseline (speedup 1.0000x reference)
"""BiLSTM-CRF loss kernel for Trainium2 (8 NeuronCores, data-parallel over batch).

Design (per core, B_loc=16 sequences):
  - All state kept transposed: hidden dim on partitions, batch on free dim.
  - LSTM recurrence: weights-stationary matmuls (8 gate-chunks x 2 K-tiles,
    N=16 batch streaming), per-step masking via copy_predicated with a
    DMA-broadcast mask-replica tile.
  - Input projection x @ W_ih^T computed on the fly in 32-step windows
    (embedding gather -> PE transpose -> N=512 matmuls), never hits DRAM.
  - Emissions computed incrementally (2 small matmuls per step/direction)
    into a (20, T*16) SBUF buffer.
  - CRF log-partition via the *backward* (beta) recursion in exp space,
    folded into the backward-LSTM phase step by step; periodic per-column
    rescaling (compensated in log space) keeps fp32 in range.
  - Gold-path score: unary via host-built one-hot mask x emit reduce;
    transition term via indirect row-gather of `transition` by tags.
"""

import numpy as np

PAD_IDX = 0
VOCAB, K, E, H = 30000, 20, 256, 256
B, T = 128, 512
NCORES = 8
BL = B // NCORES          # 16 sequences per core
WIN = 32                  # proj window (time steps)
NW = T // WIN             # 16 windows
RESCALE = 8               # CRF rescale interval

_cache = {}


def _build_program(dt_w):
    """Build the SPMD Bass program. dt_w: matmul weight/stream dtype."""
    from contextlib import ExitStack
    import concourse.bass as bass
    import concourse.bacc as bacc
    import concourse.tile as tile
    from concourse import mybir
    from concourse.masks import make_identity

    f32 = mybir.dt.float32
    i32 = mybir.dt.int32

    nc = bacc.Bacc(None, target_bir_lowering=False, debug=False)
    names = {}

    with ExitStack() as ctx:
        tc = ctx.enter_context(tile.TileContext(nc))
        dram = ctx.enter_context(tc.tile_pool(name="dram", bufs=1, space="DRAM"))

        def din(key, shape, dt=f32):
            t = dram.tile(shape, dt, kind="ExternalInput", name=key)
            names[key] = t.tensor.name
            return t

        emb = din("emb", [VOCAB, E])
        toks = din("toks", [T * BL, 1], i32)          # window-major token ids
        maskf = din("maskf", [1, T * BL])             # col = t*16+b
        masku = din("masku", [1, T * BL], mybir.dt.uint8)
        tags1h = din("tags1h", [K, T * BL], mybir.dt.uint8)  # one-hot(tag) * mask
        tagsnx = din("tagsnx", [T * BL, K], mybir.dt.uint8)  # shifted one-hot * mask
        tagsfl = din("tagsfl", [T * BL, 1], i32)      # tag ids, b-major
        wih = {d: din(f"wih_{d}", [E, 4 * H], dt_w) for d in "fb"}
        whh = {d: din(f"whh_{d}", [E, 4 * H], dt_w) for d in "fb"}
        bih = {d: din(f"bih_{d}", [128, 8]) for d in "fb"}
        woutT = din("woutT", [4, 128, K], dt_w)       # chunks: Fk0,Fk1,Bk0,Bk1
        bout = din("bout", [K, 1])
        transT = din("transT", [K, K])                # transition.T
        trans = din("trans", [K, K])                  # raw, for row gather
        out_loss = dram.tile([1, BL], f32, kind="ExternalOutput")
        names["out"] = out_loss.tensor.name

        sg = ctx.enter_context(tc.tile_pool(name="sg", bufs=1))       # singles
        tmp = ctx.enter_context(tc.tile_pool(name="tmp", bufs=4))     # step temps
        gat = ctx.enter_context(tc.tile_pool(name="gat", bufs=8))     # gather tiles
        winp = ctx.enter_context(tc.tile_pool(name="winp", bufs=2))   # xw windows
        xtw = ctx.enter_context(tc.tile_pool(name="xtw", bufs=3))
        fin = ctx.enter_context(tc.tile_pool(name="fin", bufs=3))     # finalize
        ps_g = ctx.enter_context(tc.tile_pool(name="ps_g", bufs=2, space="PSUM"))
        ps_w = ctx.enter_context(tc.tile_pool(name="ps_w", bufs=1, space="PSUM"))
        ps_t = ctx.enter_context(tc.tile_pool(name="ps_t", bufs=1, space="PSUM"))
        ps_s = ctx.enter_context(tc.tile_pool(name="ps_s", bufs=4, space="PSUM"))

        # ---- resident SBUF tensors ----
        s_wih = {d: sg.tile([128, 2, 4 * H], dt_w, tag=f"wih{d}", name=f"wih{d}") for d in "fb"}
        s_whh = {d: sg.tile([128, 2, 4 * H], dt_w, tag=f"whh{d}", name=f"whh{d}") for d in "fb"}
        for d in "fb":
            nc.sync.dma_start(out=s_wih[d][:], in_=wih[d][:].rearrange("(k p) m -> p k m", p=128))
            nc.sync.dma_start(out=s_whh[d][:], in_=whh[d][:].rearrange("(k p) m -> p k m", p=128))
        s_bih = {d: sg.tile([128, 8], f32, tag=f"bih{d}", name=f"bih{d}") for d in "fb"}
        for d in "fb":
            nc.sync.dma_start(out=s_bih[d][:], in_=bih[d][:])
        s_wout = sg.tile([128, 4, K], dt_w, tag="wout")
        nc.sync.dma_start(out=s_wout[:], in_=woutT[:].rearrange("c p k -> p c k"))
        s_bout = sg.tile([K, 1], f32, tag="bout")
        nc.sync.dma_start(out=s_bout[:], in_=bout[:])
        s_transT = sg.tile([K, K], f32, tag="transT")
        nc.sync.dma_start(out=s_transT[:], in_=transT[:])
        s_expAT = sg.tile([K, K], f32, tag="expAT")
        nc.scalar.activation(s_expAT[:], s_transT[:], mybir.ActivationFunctionType.Exp)

        ones = sg.tile([128, K], f32, tag="ones")
        nc.vector.memset(ones[:], 1.0)
        ident = sg.tile([128, 128], f32, tag="ident")
        make_identity(nc, ident[:])

        # mask replica: (128, T, BL), col = t*16+b, broadcast across partitions
        maskrep = sg.tile([128, T, BL], mybir.dt.uint8, tag="maskrep")
        nc.sync.dma_start(
            out=maskrep[:],
            in_=bass.AP(tensor=masku.tensor, offset=masku[:].offset,
                        ap=[[0, 128], [BL, T], [1, BL]]),
        )
        maskrow = sg.tile([1, T, BL], f32, tag="maskrow")
        nc.sync.dma_start(out=maskrow[:],
                          in_=bass.AP(tensor=maskf.tensor, offset=maskf[:].offset,
                                      ap=[[0, 1], [BL, T], [1, BL]]))

        emit = sg.tile([K, T, BL], f32, tag="emit")

        # all gather indices resident (one upfront DMA each)
        NT128 = T * BL // 128
        idxall = sg.tile([128, NT128], i32, tag="idxall")
        nc.sync.dma_start(out=idxall[:],
                          in_=bass.AP(tensor=toks.tensor, offset=toks[:].offset,
                                      ap=[[1, 128], [128, NT128]]))
        idxtag = sg.tile([128, NT128], i32, tag="idxtag")
        nc.sync.dma_start(out=idxtag[:],
                          in_=bass.AP(tensor=tagsfl.tensor, offset=tagsfl[:].offset,
                                      ap=[[1, 128], [128, NT128]]))
        s_t1h = sg.tile([K, T, BL], mybir.dt.uint8, tag="s_t1h")
        nc.sync.dma_start(out=s_t1h[:].rearrange("k t b -> k (t b)"), in_=tags1h[:])
        s_tnx = sg.tile([128, NT128, K], mybir.dt.uint8, tag="s_tnx")
        nc.sync.dma_start(out=s_tnx[:],
                          in_=tagsnx[:].rearrange("(n p) k -> p n k", p=128))

        # LSTM states (h in dt_w for matmul rhs, c in f32)
        st_h = {d: sg.tile([128, 2, BL], dt_w, tag=f"h{d}", name=f"h{d}") for d in "fb"}
        st_c = {d: sg.tile([128, 2, BL], f32, tag=f"c{d}", name=f"c{d}") for d in "fb"}
        for d in "fb":
            nc.vector.memset(st_h[d][:], 0.0)
            nc.vector.memset(st_c[d][:], 0.0)

        # CRF beta state (exp space) + log-scale accumulator
        Bv = sg.tile([K, BL], f32, tag="Bv")
        nc.vector.memset(Bv[:], 1.0)
        Lacc = sg.tile([1, BL], f32, tag="Lacc")
        nc.vector.memset(Lacc[:], 0.0)

        AF = mybir.ActivationFunctionType
        OP = mybir.AluOpType

        NG, GB = 2, BL // 2

        def mask_ap(t, parts, reps, g=None):
            """maskrep[:parts, t, cols] replicated reps times along a middle dim."""
            cs = slice(g * GB, (g + 1) * GB) if g is not None else slice(0, BL)
            base = maskrep[0:parts, t, cs]
            if reps == 1:
                return base
            return bass.AP(tensor=base.tensor, offset=base.offset,
                           ap=[base.ap[0], [0, reps], [1, cs.stop - cs.start]])

        def make_window(w, d):
            """Gather+transpose+project window w for direction d.
            Returns xw window tile (128, 8, BL, WIN) with bias folded."""
            xT = xtw.tile([128, 2, 512], dt_w, tag="xT")
            for g in range(4):
                j = w * 4 + g
                xg = gat.tile([128, E], f32, tag="xg")
                nc.gpsimd.indirect_dma_start(
                    out=xg[:], out_offset=None, in_=emb[:],
                    in_offset=bass.IndirectOffsetOnAxis(ap=idxall[:, j:j + 1], axis=0),
                )
                xg2 = gat.tile([128, E], f32, tag="xg2")
                nc.vector.tensor_copy(xg2[:], xg[:])
                for k in range(2):
                    pst = ps_t.tile([128, 128], f32, tag="pst")
                    nc.tensor.transpose(out=pst[:], in_=xg2[:, k * 128:(k + 1) * 128], identity=ident[:])
                    nc.vector.tensor_copy(xT[:, k, g * 128:(g + 1) * 128], pst[:])
            win = winp.tile([128, 8, BL, WIN], f32, tag=f"win{d}", name=f"win{d}")
            for m in range(8):
                psw = ps_w.tile([128, 512], f32, tag="psw")
                for k in range(2):
                    nc.tensor.matmul(psw[:], lhsT=s_wih[d][:, k, m * 128:(m + 1) * 128],
                                     rhs=xT[:, k, :], start=(k == 0), stop=(k == 1))
                nc.vector.tensor_scalar_add(win[:, m], psw[:], s_bih[d][:, m:m + 1])
            return win

        def lstm_mm(d, t):
            """Full-width recurrence matmuls (both groups share LDWEIGHTS)."""
            h = st_h[d]
            psg = ps_g.tile([128, 8, BL], f32, tag="psg", name="psg", bufs=2)
            for m in range(8):
                for k in range(2):
                    nc.tensor.matmul(psg[:, m], lhsT=s_whh[d][:, k, m * 128:(m + 1) * 128],
                                     rhs=h[:, k, :], start=(k == 0), stop=(k == 1))
            return psg

        def lstm_stepA(d, t, win, psg, g):
            """Gate add + activations for group g."""
            cs = slice(g * GB, (g + 1) * GB)
            toff = t % WIN
            gates = tmp.tile([128, 8, GB], f32, tag=f"gates{g}", name=f"gates{g}")
            nc.vector.tensor_tensor(gates[:], psg[:, :, cs], win[:, :, cs, toff], op=OP.add)
            gf = gates[:].rearrange("p m b -> p (m b)")
            nc.scalar.activation(gf[:, 0:4 * GB], gf[:, 0:4 * GB], AF.Sigmoid)
            nc.scalar.activation(gf[:, 4 * GB:6 * GB], gf[:, 4 * GB:6 * GB], AF.Tanh)
            nc.scalar.activation(gf[:, 6 * GB:8 * GB], gf[:, 6 * GB:8 * GB], AF.Sigmoid)
            return gates

        def lstm_stepB(d, t, gates, emit_mode, g):
            """Cell update for group g."""
            cs = slice(g * GB, (g + 1) * GB)
            h, c = st_h[d][:, :, cs], st_c[d][:, :, cs]
            gi, gff, gg, go = (gates[:, 0:2], gates[:, 2:4], gates[:, 4:6], gates[:, 6:8])
            cc = tmp.tile([128, 2, GB], f32, tag=f"cc{g}", name=f"cc{g}")
            ig = tmp.tile([128, 2, GB], f32, tag=f"ig{g}", name=f"ig{g}")
            nc.gpsimd.tensor_tensor(ig[:], gi, gg, op=OP.mult)
            nc.vector.tensor_tensor(cc[:], gff, c, op=OP.mult)
            nc.vector.tensor_tensor(cc[:], cc[:], ig[:], op=OP.add)
            m2 = mask_ap(t, 128, 2, g)
            nc.vector.copy_predicated(c, m2, cc[:])
            th = tmp.tile([128, 2, GB], f32, tag=f"th{g}", name=f"th{g}")
            nc.scalar.activation(th[:], cc[:], AF.Tanh)
            hh = tmp.tile([128, 2, GB], dt_w, tag=f"hh{g}", name=f"hh{g}")
            nc.vector.tensor_tensor(hh[:], go, th[:], op=OP.mult)
            nc.vector.copy_predicated(h, m2, hh[:])

        def emit_step(d, t, emit_mode):
            h = st_h[d]
            pse = ps_s.tile([K, BL], f32, tag="pssm", name="pse")
            cbase = 0 if d == "f" else 2
            for k in range(2):
                nc.tensor.matmul(pse[:], lhsT=s_wout[:, cbase + k, :], rhs=h[:, k, :],
                                 start=(k == 0), stop=(k == 1))
            if emit_mode == "f":
                nc.vector.tensor_scalar_add(emit[:, t, :], pse[:], s_bout[:, 0:1])
            else:
                nc.vector.tensor_tensor(emit[:, t, :], pse[:], emit[:, t, :], op=OP.add)

        # warm-up matmuls: make PE's clock pass every weight-producing op so
        # steady-state matmuls carry at most one semaphore wait
        for wt in [s_wih["f"][:, 0, 0:1], s_wih["b"][:, 0, 0:1],
                   s_whh["f"][:, 0, 0:1], s_whh["b"][:, 0, 0:1],
                   s_wout[:, 0, 0:1]]:
            psd = ps_s.tile([1, 1], f32, tag="pssm")
            nc.tensor.matmul(psd[:], lhsT=wt, rhs=wt, start=True, stop=True)
        psd = ps_s.tile([1, 1], f32, tag="pssm")
        nc.tensor.matmul(psd[:], lhsT=s_expAT[0:K, 0:1], rhs=s_expAT[0:K, 0:1], start=True, stop=True)
        psd = ps_s.tile([1, 1], f32, tag="pssm")
        nc.tensor.matmul(psd[:], lhsT=ident[:, 0:1], rhs=ident[:, 0:1], start=True, stop=True)

        # ---------------- forward phase ----------------
        for w in range(NW):
            win = make_window(w, "f")
            for t in range(w * WIN, (w + 1) * WIN):
                psg = lstm_mm("f", t)
                gts = [lstm_stepA("f", t, win, psg, g) for g in range(NG)]
                for g in range(NG):
                    lstm_stepB("f", t, gts[g], "f", g)
                emit_step("f", t, "f")

        # ---------------- backward phase + CRF beta ----------------
        expE_prev = [None, None]
        for w in range(NW - 1, -1, -1):
            win = make_window(w, "b")
            for t in range((w + 1) * WIN - 1, w * WIN - 1, -1):
                psg = lstm_mm("b", t)
                gts = [lstm_stepA("b", t, win, psg, g) for g in range(NG)]
                for g in range(NG):
                    lstm_stepB("b", t, gts[g], "b", g)
                emit_step("b", t, "b")
                for g in range(NG):
                    cs = slice(g * GB, (g + 1) * GB)
                    expE = tmp.tile([K, GB], f32, tag=f"expE{g}", name=f"expE{g}")
                    nc.scalar.activation(expE[:], emit[:, t, cs], AF.Exp)
                    if t < T - 1:
                        bp = tmp.tile([K, GB], f32, tag=f"bp{g}", name=f"bp{g}")
                        nc.vector.tensor_tensor(bp[:], Bv[:, cs], expE_prev[g][:], op=OP.mult)
                        psb = ps_s.tile([K, GB], f32, tag="pssm", name="psb")
                        nc.tensor.matmul(psb[:], lhsT=s_expAT[:], rhs=bp[:], start=True, stop=True)
                        nc.vector.copy_predicated(Bv[:, cs], mask_ap(t + 1, K, 1, g), psb[:])
                    expE_prev[g] = expE
                if t < T - 1 and t % RESCALE == 0 and t > 0:
                    pss = ps_s.tile([1, BL], f32, tag="pssm", name="pss")
                    nc.tensor.matmul(pss[:], lhsT=ones[0:K, 0:1], rhs=Bv[:], start=True, stop=True)
                    rr = tmp.tile([1, BL], f32, tag="rr")
                    nc.vector.reciprocal(rr[:], pss[:])
                    psr = ps_s.tile([K, BL], f32, tag="pssm", name="psr")
                    nc.tensor.matmul(psr[:], lhsT=ones[0:1, 0:K], rhs=rr[:], start=True, stop=True)
                    sc = tmp.tile([K, BL], f32, tag="sc")
                    nc.vector.tensor_tensor(sc[:], Bv[:], psr[:], op=OP.mult)
                    nc.vector.copy_predicated(Bv[:], mask_ap(t, K, 1), sc[:])
                    lns = tmp.tile([1, BL], f32, tag="lns")
                    nc.scalar.activation(lns[:], pss[:], AF.Ln)
                    nc.vector.tensor_tensor(lns[:], lns[:], maskrow[0:1, t, :], op=OP.mult)
                    nc.vector.tensor_tensor(Lacc[:], Lacc[:], lns[:], op=OP.add)

        # ---------------- finalize ----------------
        # log partition: logZ = ln(sum_i expE_0 * Bv_0) + Lacc
        zt = fin.tile([K, BL], f32, tag="zt")
        for g in range(NG):
            cs = slice(g * GB, (g + 1) * GB)
            nc.vector.tensor_tensor(zt[:, cs], Bv[:, cs], expE_prev[g][:], op=OP.mult)
        psz = ps_s.tile([1, BL], f32, tag="pssm")
        nc.tensor.matmul(psz[:], lhsT=ones[0:K, 0:1], rhs=zt[:], start=True, stop=True)
        logZ = fin.tile([1, BL], f32, tag="logZ")
        nc.scalar.activation(logZ[:], psz[:], AF.Ln)
        nc.vector.tensor_tensor(logZ[:], logZ[:], Lacc[:], op=OP.add)

        # unary gold score: sum over (j,t) of tags1h * emit, keep b
        Uacc = fin.tile([K, BL], f32, tag="Uacc")
        nc.vector.memset(Uacc[:], 0.0)
        CH = 32
        TC = T // CH
        for ci in range(CH):
            t1 = fin.tile([K, TC * BL], f32, tag="t1")
            nc.vector.tensor_copy(t1[:], s_t1h[:, ci * TC:(ci + 1) * TC, :].rearrange("p t b -> p (t b)"))
            um = fin.tile([K, TC * BL], f32, tag="um")
            nc.vector.tensor_tensor(
                um[:], t1[:], emit[:, ci * TC:(ci + 1) * TC, :].rearrange("p t b -> p (t b)"),
                op=OP.mult)
            ur = fin.tile([K, BL], f32, tag="ur")
            umr = bass.AP(tensor=um.tensor, offset=um[:].offset,
                          ap=[um[:].ap[0], [1, BL], [BL, TC]])
            nc.vector.tensor_reduce(ur[:], umr, axis=mybir.AxisListType.X, op=OP.add)
            nc.vector.tensor_tensor(Uacc[:], Uacc[:], ur[:], op=OP.add)
        psu = ps_s.tile([1, BL], f32, tag="pssm")
        nc.tensor.matmul(psu[:], lhsT=ones[0:K, 0:1], rhs=Uacc[:], start=True, stop=True)
        score = fin.tile([1, BL], f32, tag="score")
        nc.vector.tensor_copy(score[:], psu[:])

        # transition gold score via row gather
        QT = T // 128
        TRbuf = fin.tile([128, NT128], f32, tag="TRbuf")
        for i in range(NT128):
            tr = gat.tile([128, K], f32, tag="tr")
            nc.gpsimd.indirect_dma_start(
                out=tr[:], out_offset=None, in_=trans[:],
                in_offset=bass.IndirectOffsetOnAxis(ap=idxtag[:, i:i + 1], axis=0))
            sel = gat.tile([128, K], f32, tag="sel")
            nc.vector.tensor_copy(sel[:], s_tnx[:, i, :])
            nc.vector.tensor_tensor(tr[:], tr[:], sel[:], op=OP.mult)
            nc.vector.tensor_reduce(TRbuf[:, i:i + 1], tr[:], axis=mybir.AxisListType.X, op=OP.add)
        pstr = ps_s.tile([1, NT128], f32, tag="pssm")
        nc.tensor.matmul(pstr[:], lhsT=ones[:, 0:1], rhs=TRbuf[:], start=True, stop=True)
        trv = fin.tile([1, BL], f32, tag="trv")
        ptr_ap = bass.AP(tensor=pstr.tensor, offset=pstr[:].offset,
                         ap=[pstr[:].ap[0], [QT, BL], [1, QT]])
        nc.vector.tensor_reduce(trv[:], ptr_ap, axis=mybir.AxisListType.X, op=OP.add)

        # loss = logZ - (score + trans)
        nc.vector.tensor_tensor(score[:], score[:], trv[:], op=OP.add)
        res = fin.tile([1, BL], f32, tag="res")
        nc.vector.tensor_tensor(res[:], logZ[:], score[:], op=OP.subtract)
        nc.sync.dma_start(out=out_loss[:], in_=res[:])

    nc.compile()
    return nc, names


def _prep_core(inputs, k, dt_np):
    """Build the per-core input map (host-side index plumbing only)."""
    s = slice(k * BL, (k + 1) * BL)
    sent = np.asarray(inputs["sentences"][s])          # (16, 512) i32
    tags = np.asarray(inputs["tags"][s])               # (16, 512) i32
    mask = (sent != PAD_IDX)
    maskf = mask.T.astype(np.float32).reshape(1, T * BL)       # col=t*16+b
    toks = sent.reshape(BL, NW, WIN).transpose(1, 0, 2).reshape(T * BL, 1)
    oh = (tags[:, :, None] == np.arange(K)[None, None, :])
    tags1h = (oh & mask[:, :, None]).transpose(2, 1, 0).reshape(K, T * BL)
    tnx = np.zeros((BL, T, K), np.float32)
    tnx[:, :-1, :] = (oh[:, 1:, :] & mask[:, 1:, None]).astype(np.float32)
    m = {
        "toks": toks.astype(np.int32),
        "maskf": maskf,
        "masku": mask.T.astype(np.uint8).reshape(1, T * BL),
        "tags1h": tags1h.astype(np.uint8),
        "tagsnx": tnx.reshape(T * BL, K).astype(np.uint8),
        "tagsfl": tags.reshape(T * BL, 1).astype(np.int32),
        "emb": np.asarray(inputs["embedding"], np.float32),
        "wih_f": np.ascontiguousarray(np.asarray(inputs["w_ih_f"]).T).astype(dt_np),
        "wih_b": np.ascontiguousarray(np.asarray(inputs["w_ih_b"]).T).astype(dt_np),
        "whh_f": np.ascontiguousarray(np.asarray(inputs["w_hh_f"]).T).astype(dt_np),
        "whh_b": np.ascontiguousarray(np.asarray(inputs["w_hh_b"]).T).astype(dt_np),
        "bih_f": np.ascontiguousarray(np.asarray(inputs["b_f"]).reshape(8, 128).T).astype(np.float32),
        "bih_b": np.ascontiguousarray(np.asarray(inputs["b_b"]).reshape(8, 128).T).astype(np.float32),
        "woutT": np.ascontiguousarray(np.asarray(inputs["w_out"]).T.reshape(4, 128, K)).astype(dt_np),
        "bout": np.asarray(inputs["b_out"]).reshape(K, 1).astype(np.float32),
        "transT": np.ascontiguousarray(np.asarray(inputs["transition"]).T).astype(np.float32),
        "trans": np.asarray(inputs["transition"], np.float32),
    }
    return m


def kernel(**inputs):
    import ml_dtypes
    from concourse import mybir
    from concourse.bass_utils import run_bass_kernel_spmd

    use_bf16 = _cache.get("use_bf16", True)
    key = ("prog", use_bf16)
    if key not in _cache:
        dt_w = mybir.dt.bfloat16 if use_bf16 else mybir.dt.float32
        _cache[key] = _build_program(dt_w)
    nc, names = _cache[key]
    dt_np = ml_dtypes.bfloat16 if use_bf16 else np.float32

    in_maps = []
    for k in range(NCORES):
        m = _prep_core(inputs, k, dt_np)
        in_maps.append({names[kk]: vv for kk, vv in m.items()})

    res = run_bass_kernel_spmd(nc, in_maps, core_ids=list(range(NCORES)),
                               **_cache.get("run_kwargs", {}))
    out = np.concatenate([r[names["out"]].reshape(BL) for r in res.results])
    _cache["last_results"] = res
    return out.astype(np.float32)



# revision 38
# speedup vs baseline: 2.0252x; 2.0252x over previous
"""BiLSTM-CRF loss kernel for Trainium2 (8 NeuronCores, data-parallel over batch).

v2 design (per core, BL=16 sequences):
  Phase 1 (512 wall-steps): fwd LSTM (t=s) and bwd LSTM (t=511-s) run as two
    interleaved streams so every engine pipelines across streams.
    - Gate order host-permuted to [i,f,o,g] so one Sigmoid op covers i,f,o.
    - Input projection xw is windowed (32 steps) and injected into the gate
      PSUM accumulation via an identity matmul (PE is cheap, DVE is not).
    - Emissions pre-biased once; both streams add their w_out matmul via a
      single joint DVE op per step.
    - Only Sigmoid/Tanh on ACT -> zero activation-table thrash.
    - Gold-score (unary+transition) reductions interleaved into phase-1 slack.
  Phase 2 (~256 wall-steps): CRF log-partition via two interleaved exp-space
    recursions on bulk-exp'd emissions: alpha upward (t=1..256) and beta
    downward (t=510..256), meeting at t*=256. Periodic rescale; the log
    compensation terms are stored and Ln'd once at the end.
"""

import numpy as np

PAD_IDX = 0
VOCAB, K, E, H = 30000, 20, 256, 256
B, T = 128, 512
NCORES = 8
BL = B // NCORES          # 16 sequences per core
WIN = 32                  # proj window (time steps)
NW = T // WIN             # 16 windows
RESC = 8                  # CRF rescale interval (wall steps)
TME = 256                 # CRF meeting point: logZ = ln(sum D_TME * B_TME)
NRESC = 31                # rescales at k=8,16,...,248

_cache = {}


def _build_program():
    from contextlib import ExitStack
    import concourse.bass as bass
    import concourse.bacc as bacc
    import concourse.tile as tile
    from concourse import mybir
    from concourse.masks import make_identity

    f32 = mybir.dt.float32
    bf16 = mybir.dt.bfloat16
    i32 = mybir.dt.int32
    u8 = mybir.dt.uint8
    AF = mybir.ActivationFunctionType
    OP = mybir.AluOpType

    nc = bacc.Bacc(None, target_bir_lowering=False, debug=False)
    names = {}

    with ExitStack() as ctx:
        tc = ctx.enter_context(tile.TileContext(nc))
        dram = ctx.enter_context(tc.tile_pool(name="dram", bufs=1, space="DRAM"))

        def din(key, shape, dt=f32):
            t = dram.tile(shape, dt, kind="ExternalInput", name=key)
            names[key] = t.tensor.name
            return t

        emb = din("emb", [VOCAB, E], bf16)
        toks = din("toks", [T * BL, 1], i32)            # window-major token ids
        masku = din("masku", [1, T * BL], u8)           # col = t*16+b
        t1h = din("t1h", [K, T * BL], bf16)             # one-hot(tag)*mask
        tnx = din("tnx", [K, T * BL], bf16)             # shifted one-hot*mask
        lnmask = din("lnmask", [1, (NRESC + 1) * 2 * BL])  # mask at rescale times
        wih = {d: din(f"wih_{d}", [E, 4 * H], bf16) for d in "fb"}
        whh = {d: din(f"whh_{d}", [E, 4 * H], bf16) for d in "fb"}
        bih = {d: din(f"bih_{d}", [128, 8]) for d in "fb"}
        bihT = {d: din(f"bihT_{d}", [1, 8 * 128], bf16) for d in "fb"}
        woutT = din("woutT", [4, 128, K], bf16)         # chunks: Fk0,Fk1,Bk0,Bk1
        bout = din("bout", [K, 1])
        expA = din("expA", [K, K], bf16)                # exp(transition)
        expAT = din("expAT", [K, K], bf16)              # exp(transition).T
        Abf = din("Abf", [K, K], bf16)                  # transition (bf16)
        wstar = din("wstar", [1, K])                    # 1^T (expA^T)^{-1}
        out_loss = dram.tile([1, BL], f32, kind="ExternalOutput")
        names["out"] = out_loss.tensor.name

        sg = ctx.enter_context(tc.tile_pool(name="sg", bufs=1))       # singles
        tmp = ctx.enter_context(tc.tile_pool(name="tmp", bufs=4))     # step temps
        gat = ctx.enter_context(tc.tile_pool(name="gat", bufs=4))     # gather tiles
        fin = ctx.enter_context(tc.tile_pool(name="fin", bufs=3))     # finalize
        ps_g = ctx.enter_context(tc.tile_pool(name="ps_g", bufs=2, space="PSUM"))
        ps_w = ctx.enter_context(tc.tile_pool(name="ps_w", bufs=2, space="PSUM"))
        ps_t = ctx.enter_context(tc.tile_pool(name="ps_t", bufs=1, space="PSUM"))
        ps_s = ctx.enter_context(tc.tile_pool(name="ps_s", bufs=2, space="PSUM"))

        # ---- resident SBUF tensors ----
        s_wih = {d: sg.tile([128, 2, 4 * H], bf16, tag=f"wih{d}", name=f"wih{d}") for d in "fb"}
        s_whh = {d: sg.tile([128, 2, 4 * H], bf16, tag=f"whh{d}", name=f"whh{d}") for d in "fb"}
        for d in "fb":
            nc.sync.dma_start(out=s_wih[d][:], in_=wih[d][:].rearrange("(k p) m -> p k m", p=128))
            nc.sync.dma_start(out=s_whh[d][:], in_=whh[d][:].rearrange("(k p) m -> p k m", p=128))
        s_bih = {d: sg.tile([128, 8], f32, tag=f"bih{d}", name=f"bih{d}") for d in "fb"}
        s_bihT = {d: sg.tile([1, 8, 128], bf16, tag=f"bihT{d}", name=f"bihT{d}") for d in "fb"}
        for d in "fb":
            nc.sync.dma_start(out=s_bih[d][:], in_=bih[d][:])
            nc.sync.dma_start(out=s_bihT[d][:].rearrange("o m p -> o (m p)"), in_=bihT[d][:])
        onesb = sg.tile([1, 512], bf16, tag="onesb")
        nc.vector.memset(onesb[:], 1.0)
        s_wout = sg.tile([128, 4, K], bf16, tag="wout")
        nc.sync.dma_start(out=s_wout[:], in_=woutT[:].rearrange("c p k -> p c k"))
        s_bout = sg.tile([K, 1], f32, tag="bout")
        nc.sync.dma_start(out=s_bout[:], in_=bout[:])
        s_eA = sg.tile([K, K], bf16, tag="eA")
        nc.sync.dma_start(out=s_eA[:], in_=expA[:])
        s_eAT = sg.tile([K, K], bf16, tag="eAT")
        nc.sync.dma_start(out=s_eAT[:], in_=expAT[:])
        s_A = sg.tile([K, K], bf16, tag="A")
        nc.sync.dma_start(out=s_A[:], in_=Abf[:])
        s_ws = sg.tile([1, K], f32, tag="ws")
        nc.sync.dma_start(out=s_ws[:], in_=wstar[:])
        s_t1h = sg.tile([K, T * BL], bf16, tag="t1h")
        nc.sync.dma_start(out=s_t1h[:], in_=t1h[:])
        s_tnx = sg.tile([K, T * BL], bf16, tag="tnx")
        nc.sync.dma_start(out=s_tnx[:], in_=tnx[:])
        s_lnm = sg.tile([1, (NRESC + 1) * 2 * BL], f32, tag="lnm")
        nc.sync.dma_start(out=s_lnm[:], in_=lnmask[:])

        ones = sg.tile([128, K], f32, tag="ones")
        nc.vector.memset(ones[:], 1.0)
        ident = sg.tile([128, 128], bf16, tag="ident")
        make_identity(nc, ident[:])

        # mask replica: (128, T, BL), col = t*16+b, broadcast across partitions
        maskrep = sg.tile([128, T, BL], u8, tag="maskrep")
        nc.sync.dma_start(
            out=maskrep[:],
            in_=bass.AP(tensor=masku.tensor, offset=masku[:].offset,
                        ap=[[0, 128], [BL, T], [1, BL]]),
        )

        # token indices for gathers (128 per column)
        NT128 = T * BL // 128
        idxall = sg.tile([128, NT128], i32, tag="idxall")
        nc.sync.dma_start(out=idxall[:],
                          in_=bass.AP(tensor=toks.tensor, offset=toks[:].offset,
                                      ap=[[1, 128], [128, NT128]]))

        # emissions (f32) and their exp; pre-bias emit with b_out
        emit = sg.tile([K, T, BL], f32, tag="emit")
        emf0 = emit[:].rearrange("k t b -> k (t b)")
        nc.vector.memset(emf0, 0.0)
        nc.vector.tensor_scalar_add(emf0, emf0, s_bout[:, 0:1])
        expE = emit  # exp taken in place after phase 1 (unary reads are done by then)

        # per-direction rotating transposed-input windows and projection windows
        xtp = ctx.enter_context(tc.tile_pool(name="xtp", bufs=2))
        winp = ctx.enter_context(tc.tile_pool(name="winp", bufs=2))

        # LSTM states, layout [128, dir(2), k(2), BL]
        st_h = sg.tile([128, 2, 2, BL], bf16, tag="st_h")
        st_c = sg.tile([128, 2, 2, BL], f32, tag="st_c")
        nc.vector.memset(st_h[:], 0.0)
        nc.vector.memset(st_c[:], 0.0)

        # CRF state S[:, 0]=alpha D, S[:, 1]=beta B; ln-compensation buffer
        S = sg.tile([K, 2, BL], f32, tag="S")
        lnbuf = sg.tile([1, NRESC + 1, 2, BL], f32, tag="lnbuf")
        Uacc = sg.tile([K, BL], f32, tag="Uacc")
        nc.vector.memset(Uacc[:], 0.0)
        TRacc = sg.tile([K, BL], f32, tag="TRacc")
        nc.vector.memset(TRacc[:], 0.0)

        def gather_window(w, d):
            """Gather+transpose window w into a rotating per-direction xT tile."""
            xT = xtp.tile([128, 2, 512], bf16, tag=f"xT{d}", name=f"xT{d}")
            for g in range(4):
                j = w * 4 + g
                xg = gat.tile([128, E], bf16, tag="xg")
                nc.gpsimd.indirect_dma_start(
                    out=xg[:], out_offset=None, in_=emb[:],
                    in_offset=bass.IndirectOffsetOnAxis(ap=idxall[:, j:j + 1], axis=0),
                )
                for k in range(2):
                    pst = ps_t.tile([128, 128], bf16, tag="pst")
                    nc.tensor.transpose(out=pst[:], in_=xg[:, k * 128:(k + 1) * 128],
                                        identity=ident[:])
                    if (g + k) % 2 == 0:
                        nc.vector.tensor_copy(xT[:, k, g * 128:(g + 1) * 128], pst[:])
                    else:
                        nc.scalar.activation(xT[:, k, g * 128:(g + 1) * 128], pst[:], AF.Copy)
            return xT

        def make_window(xT, d):
            """Project a gathered window for direction d -> win (128, 8, BL, WIN)
            bf16, bias folded via an extra matmul. psw column order: (b, t)."""
            win = winp.tile([128, 8, BL, WIN], bf16, tag=f"win{d}", name=f"win{d}")
            for m in range(8):
                psw = ps_w.tile([128, 512], f32, tag="psw")
                for k in range(2):
                    nc.tensor.matmul(psw[:], lhsT=s_wih[d][:, k, m * 128:(m + 1) * 128],
                                     rhs=xT[:, k, :], start=(k == 0), stop=False)
                nc.tensor.matmul(psw[:], lhsT=s_bihT[d][:, m, :], rhs=onesb[:],
                                 start=False, stop=True)
                wv = win[:, m].rearrange("p b t -> p (b t)")
                if m % 2 == 0:
                    nc.scalar.activation(wv, psw[:], AF.Copy)
                else:
                    nc.vector.tensor_copy(wv, psw[:])
            return win

        # mask AP helpers --------------------------------------------------
        def mask_joint(tlo, thi, parts, reps):
            """(parts, 2, reps, BL) AP over maskrep: dir0 at t=tlo, dir1 at t=thi."""
            base = maskrep[0:parts, tlo, 0:BL]
            return bass.AP(tensor=base.tensor, offset=base.offset,
                           ap=[base.ap[0], [(thi - tlo) * BL, 2], [0, reps], [1, BL]])

        def mask_one(t, parts, reps):
            base = maskrep[0:parts, t, 0:BL]
            if reps == 1:
                return base
            return bass.AP(tensor=base.tensor, offset=base.offset,
                           ap=[base.ap[0], [0, reps], [1, BL]])

        # -------- gold-score chunk work (interleaved into phase 1) --------
        NCH = 16
        CW = T * BL // NCH          # 512 cols per chunk

        def trans_chunk(ci):
            """TRacc += reduce_t(A[tag, :] * shifted-onehot) for chunk ci."""
            psa = ps_w.tile([K, CW], f32, tag="psw", name="psa")
            nc.tensor.matmul(psa[:], lhsT=s_A[:], rhs=s_t1h[:, ci * CW:(ci + 1) * CW],
                             start=True, stop=True)
            um = fin.tile([K, CW], f32, tag="um")
            nc.vector.tensor_tensor(um[:], psa[:], s_tnx[:, ci * CW:(ci + 1) * CW], op=OP.mult)
            ur = fin.tile([K, BL], f32, tag="ur")
            umr = bass.AP(tensor=um.tensor, offset=um[:].offset,
                          ap=[um[:].ap[0], [1, BL], [BL, CW // BL]])
            nc.vector.tensor_reduce(ur[:], umr, axis=mybir.AxisListType.X, op=OP.add)
            nc.vector.tensor_tensor(TRacc[:], TRacc[:], ur[:], op=OP.add)

        def unary_chunk(ci):
            """Uacc += reduce_t(onehot * emit) for chunk ci (emit must be complete)."""
            um = fin.tile([K, CW], f32, tag="um")
            ef = emit[:].rearrange("k t b -> k (t b)")
            nc.gpsimd.tensor_tensor(um[:], s_t1h[:, ci * CW:(ci + 1) * CW],
                                    ef[:, ci * CW:(ci + 1) * CW], op=OP.mult)
            ur = fin.tile([K, BL], f32, tag="ur")
            umr = bass.AP(tensor=um.tensor, offset=um[:].offset,
                          ap=[um[:].ap[0], [1, BL], [BL, CW // BL]])
            nc.vector.tensor_reduce(ur[:], umr, axis=mybir.AxisListType.X, op=OP.add)
            nc.vector.tensor_tensor(Uacc[:], Uacc[:], ur[:], op=OP.add)

        # emit chunk completion wall-step: chunk ci covers t in [ci*32,(ci+1)*32)
        unary_sched = {}
        for ci in range(NCH):
            t0c, t1c = ci * (T // NCH), (ci + 1) * (T // NCH) - 1
            done = max(t1c, T - 1 - t0c)
            unary_sched.setdefault(done, []).append(ci)

        # ---------------- phase 1: interleaved fwd/bwd LSTM ----------------
        # prefetch first windows
        win_cur = {"f": make_window(gather_window(0, "f"), "f"),
                   "b": make_window(gather_window(15, "b"), "b")}
        win_nxt = {}
        xt_nxt = {}

        warm = ps_s.tile([1, 1], f32, tag="pssm")
        nc.tensor.matmul(warm[:], lhsT=ident[:, 0:1], rhs=ident[:, 0:1], start=True, stop=True)

        for s in range(T):
            blk, toff = divmod(s, WIN)
            tf, tb = s, T - 1 - s
            tlo, thi = (tf, tb) if tf < tb else (tb, tf)
            jf = 0 if tf < tb else 1    # pse slot for fwd so emit AP stride >= 0
            jb = 1 - jf

            # recurrence + window-injection matmuls
            psg = ps_g.tile([128, 2, 8, BL], f32, tag="psg", name="psg")
            for di, d in enumerate("fb"):
                t_d = tf if d == "f" else tb
                tof = toff if d == "f" else WIN - 1 - toff
                for m in range(8):
                    for k in range(2):
                        nc.tensor.matmul(psg[:, di, m], lhsT=s_whh[d][:, k, m * 128:(m + 1) * 128],
                                         rhs=st_h[:, di, k, :], start=(m == 0 and k == 0), stop=False)
                wslice = bass.AP(tensor=win_cur[d].tensor,
                                 offset=win_cur[d][:, :, :, tof].offset,
                                 ap=[win_cur[d][:].ap[0], [BL * WIN, 8], [WIN, BL]])
                nc.tensor.matmul(psg[:, di], lhsT=ident[:], rhs=wslice, start=False, stop=True)

            # activations straight from PSUM: sigmoid(i,f,o)=chunks 0..5, tanh(g)=6..7
            gates = tmp.tile([128, 2, 8, BL], f32, tag="gates", name="gates")
            nc.scalar.activation(gates[:, :, 0:6, :], psg[:, :, 0:6, :], AF.Sigmoid)
            nc.scalar.activation(gates[:, :, 6:8, :], psg[:, :, 6:8, :], AF.Tanh)

            # cell update (joint over both streams)
            ig = tmp.tile([128, 2, 2, BL], f32, tag="ig", name="ig")
            nc.gpsimd.tensor_tensor(ig[:], gates[:, :, 0:2, :], gates[:, :, 6:8, :], op=OP.mult)
            cc = tmp.tile([128, 2, 2, BL], f32, tag="cc", name="cc")
            nc.vector.tensor_tensor(cc[:], gates[:, :, 2:4, :], st_c[:], op=OP.mult)
            nc.vector.tensor_tensor(cc[:], cc[:], ig[:], op=OP.add)
            if tf < tb:
                nc.vector.copy_predicated(st_c[:], mask_joint(tf, tb, 128, 2), cc[:])
            else:
                nc.vector.tensor_copy(st_c[:, 0], cc[:, 0])
                nc.vector.copy_predicated(st_c[:, 1], mask_one(tb, 128, 2), cc[:, 1])
            th = tmp.tile([128, 2, 2, BL], f32, tag="th", name="th")
            nc.scalar.activation(th[:], cc[:], AF.Tanh)
            # h = o * tanh(c): fwd needs no masking (padded region is dont-care)
            nc.gpsimd.tensor_tensor(st_h[:, 0], gates[:, 0, 4:6, :], th[:, 0], op=OP.mult)
            hh = tmp.tile([128, 2, BL], bf16, tag="hh", name="hh")
            nc.gpsimd.tensor_tensor(hh[:], gates[:, 1, 4:6, :], th[:, 1], op=OP.mult)
            nc.vector.copy_predicated(st_h[:, 1], mask_one(tb, 128, 2), hh[:])

            # emissions: pse[:, jf] = fwd, pse[:, jb] = bwd; joint add into emit
            pse = ps_s.tile([K, 2, BL], f32, tag="pssm", name="pse")
            for k in range(2):
                nc.tensor.matmul(pse[:, jf], lhsT=s_wout[:, k, :], rhs=st_h[:, 0, k, :],
                                 start=(k == 0), stop=(k == 1))
            for k in range(2):
                nc.tensor.matmul(pse[:, jb], lhsT=s_wout[:, 2 + k, :], rhs=st_h[:, 1, k, :],
                                 start=(k == 0), stop=(k == 1))
            eap = bass.AP(tensor=emit.tensor, offset=emit[:, tlo, :].offset,
                          ap=[emit[:].ap[0], [(thi - tlo) * BL, 2], [1, BL]])
            nc.vector.tensor_tensor(eap, eap, pse[:], op=OP.add)

            # window prefetch/projection for the next block, spread over the block
            if blk < NW - 1:
                if toff == 4:
                    xt_nxt["f"] = gather_window(blk + 1, "f")
                if toff == 12:
                    xt_nxt["b"] = gather_window(NW - 2 - blk, "b")
                if toff == 20:
                    win_nxt["f"] = make_window(xt_nxt["f"], "f")
                if toff == 26:
                    win_nxt["b"] = make_window(xt_nxt["b"], "b")
                if toff == WIN - 1:
                    win_cur = dict(win_nxt)

            # interleave gold-score chunks
            if s % 32 == 16 and s // 32 < NCH // 2:
                ci = s // 32
                trans_chunk(2 * ci)
                trans_chunk(2 * ci + 1)
            for ci in unary_sched.get(s, []):
                unary_chunk(ci)

        # ---------------- phase 1.5: bulk exp ----------------
        ef = emit[:].rearrange("k t b -> k (t b)")
        xf = expE[:].rearrange("k t b -> k (t b)")
        for q in range(4):
            sl = slice(q * T * BL // 4, (q + 1) * T * BL // 4)
            nc.scalar.activation(xf[:, sl], ef[:, sl], AF.Exp)

        # ---------------- phase 2: CRF alpha/beta meeting at TME ----------------
        # Alpha runs in Q-space (Q_t = expA^T @ D_t) so both streams share the
        # "multiply by expE, then matmul" shape:
        #   alpha (dir 0): S0 <- masked_{m[ta]}  (expA^T @ (S0 * expE[ta])),  ta = 0..TME-1
        #   beta  (dir 1): S1 <- masked_{m[te]}  (expA   @ (S1 * expE[te])),  te = 511..TME+1
        # final: logZ = ln sum_i (expE[TME] * Q_{TME-1} * B_TME)[i] + ln-comp terms
        nc.vector.memset(S[:], 1.0)

        nrs = 0
        for kk in range(TME):
            ta = kk                # alpha uses expE[ta], mask[ta]
            te = T - kk            # beta uses expE[te], mask[te]; skip kk=0 (te=512)
            bp = tmp.tile([K, 2, BL], bf16, tag="bp", name="bp")
            psj = ps_s.tile([K, 2, BL], f32, tag="pssm", name="psj")
            if kk == 0:
                nc.vector.tensor_tensor(bp[:, 0], S[:, 0], expE[:, ta, :], op=OP.mult)
                nc.tensor.matmul(psj[:, 0], lhsT=s_eA[:], rhs=bp[:, 0], start=True, stop=True)
                nc.vector.copy_predicated(S[:, 0], mask_one(ta, K, 1), psj[:, 0])
                continue
            # joint: dir0 alpha reads expE[ta], dir1 beta reads expE[te]
            xap = bass.AP(tensor=expE.tensor, offset=expE[:, ta, :].offset,
                          ap=[expE[:].ap[0], [(te - ta) * BL, 2], [1, BL]])
            nc.vector.tensor_tensor(bp[:], S[:], xap, op=OP.mult)
            nc.tensor.matmul(psj[:, 0], lhsT=s_eA[:], rhs=bp[:, 0], start=True, stop=True)
            nc.tensor.matmul(psj[:, 1], lhsT=s_eAT[:], rhs=bp[:, 1], start=True, stop=True)
            nc.vector.copy_predicated(S[:], mask_joint(ta, te, K, 1), psj[:])

            if kk % RESC == 0 and kk >= 8 and kk <= 248:
                # rescale both streams; ln terms batched at the end
                pss = ps_s.tile([1, 2 * BL], f32, tag="pssm", name="pss")
                nc.tensor.matmul(pss[:], lhsT=ones[0:K, 0:1],
                                 rhs=S[:].rearrange("k d b -> k (d b)"), start=True, stop=True)
                nc.vector.tensor_copy(lnbuf[:, nrs].rearrange("o d b -> o (d b)"), pss[:])
                rr = tmp.tile([1, 2 * BL], f32, tag="rr")
                nc.vector.reciprocal(rr[:], pss[:])
                psr = ps_s.tile([K, 2 * BL], f32, tag="pssm", name="psr")
                nc.tensor.matmul(psr[:], lhsT=ones[0:1, 0:K], rhs=rr[:], start=True, stop=True)
                sc = tmp.tile([K, 2, BL], f32, tag="sc")
                nc.vector.tensor_tensor(sc[:].rearrange("k d b -> k (d b)"),
                                        S[:].rearrange("k d b -> k (d b)"), psr[:], op=OP.mult)
                nc.vector.copy_predicated(S[:], mask_joint(ta, te, K, 1), sc[:])
                nrs += 1
        assert nrs == NRESC, nrs

        # final compensated rescale (bounds the final Ln input); plain scaling,
        # the lnmask row gates S1's compensation to columns that use it
        pss = ps_s.tile([1, 2 * BL], f32, tag="pssm", name="pssf")
        nc.tensor.matmul(pss[:], lhsT=ones[0:K, 0:1],
                         rhs=S[:].rearrange("k d b -> k (d b)"), start=True, stop=True)
        nc.vector.tensor_copy(lnbuf[:, NRESC].rearrange("o d b -> o (d b)"), pss[:])
        rr = tmp.tile([1, 2 * BL], f32, tag="rr")
        nc.vector.reciprocal(rr[:], pss[:])
        psr = ps_s.tile([K, 2 * BL], f32, tag="pssm", name="psrf")
        nc.tensor.matmul(psr[:], lhsT=ones[0:1, 0:K], rhs=rr[:], start=True, stop=True)
        nc.vector.tensor_tensor(S[:].rearrange("k d b -> k (d b)"),
                                S[:].rearrange("k d b -> k (d b)"), psr[:], op=OP.mult)

        # ---------------- finalize ----------------
        # logZ = ln(sum_i D[i]*B[i]) + sum(masked ln rescale terms)
        lns = fin.tile([1, (NRESC + 1) * 2 * BL], f32, tag="lns")
        nc.scalar.activation(lns[:], lnbuf[:].rearrange("o r d b -> o (r d b)"), AF.Ln)
        nc.vector.tensor_tensor(lns[:], lns[:], s_lnm[:], op=OP.mult)
        lnred = fin.tile([1, BL], f32, tag="lnred")
        lnsr = bass.AP(tensor=lns.tensor, offset=lns[:].offset,
                       ap=[lns[:].ap[0], [1, BL], [BL, (NRESC + 1) * 2]])
        nc.vector.tensor_reduce(lnred[:], lnsr, axis=mybir.AxisListType.X, op=OP.add)

        # combine vector V: active cols (len > TME) use expE[TME]*beta,
        # frozen cols (len <= TME) use w* (recovers sum_i D_i from Q-space)
        psR = ps_s.tile([K, BL], f32, tag="pssm", name="psR")
        nc.tensor.matmul(psR[:], lhsT=s_ws[:], rhs=ones[0:1, 0:BL], start=True, stop=True)
        V = fin.tile([K, BL], f32, tag="V")
        nc.vector.tensor_copy(V[:], psR[:])
        EV = fin.tile([K, BL], f32, tag="EV")
        nc.vector.tensor_tensor(EV[:], expE[:, TME, :], S[:, 1], op=OP.mult)
        nc.vector.copy_predicated(V[:], mask_one(TME, K, 1), EV[:])
        zt = fin.tile([K, BL], f32, tag="zt")
        nc.vector.tensor_tensor(zt[:], S[:, 0], V[:], op=OP.mult)
        psz = ps_s.tile([1, BL], f32, tag="pssm", name="psz")
        nc.tensor.matmul(psz[:], lhsT=ones[0:K, 0:1], rhs=zt[:], start=True, stop=True)
        logZ = fin.tile([1, BL], f32, tag="logZ")
        nc.scalar.activation(logZ[:], psz[:], AF.Ln)
        nc.vector.tensor_tensor(logZ[:], logZ[:], lnred[:], op=OP.add)

        # gold score = colsum(Uacc) + colsum(TRacc)
        nc.vector.tensor_tensor(Uacc[:], Uacc[:], TRacc[:], op=OP.add)
        psu = ps_s.tile([1, BL], f32, tag="pssm", name="psu")
        nc.tensor.matmul(psu[:], lhsT=ones[0:K, 0:1], rhs=Uacc[:], start=True, stop=True)

        res = fin.tile([1, BL], f32, tag="res")
        nc.vector.tensor_tensor(res[:], logZ[:], psu[:], op=OP.subtract)
        nc.sync.dma_start(out=out_loss[:], in_=res[:])

    nc.compile()
    return nc, names


def _prep_core(inputs, kcore):
    """Per-core host-side input prep (index plumbing + layout shuffles)."""
    import ml_dtypes
    bf = ml_dtypes.bfloat16
    s = slice(kcore * BL, (kcore + 1) * BL)
    sent = np.asarray(inputs["sentences"][s])          # (16, 512) i32
    tags = np.asarray(inputs["tags"][s])               # (16, 512) i32
    mask = (sent != PAD_IDX)                           # (16, 512)

    # gate-row permutation [i,f,g,o] -> [i,f,o,g]
    perm = np.concatenate([np.arange(0, 2 * H), np.arange(3 * H, 4 * H),
                           np.arange(2 * H, 3 * H)])

    toks = sent.reshape(BL, NW, WIN).transpose(1, 0, 2).reshape(T * BL, 1)
    oh = (tags[:, :, None] == np.arange(K)[None, None, :])
    t1h = (oh & mask[:, :, None]).transpose(2, 1, 0).reshape(K, T * BL)
    tnxm = np.zeros((BL, T, K), np.float32)
    tnxm[:, :-1, :] = (oh[:, 1:, :] & mask[:, 1:, None]).astype(np.float32)
    tnx = tnxm.transpose(2, 1, 0).reshape(K, T * BL)

    # lnmask: mask value at the rescale checkpoints, layout (r, dir, b)
    lnm = np.zeros((NRESC + 1, 2, BL), np.float32)
    for r in range(NRESC):
        kk = 8 * (r + 1)
        lnm[r, 0] = mask[:, kk].astype(np.float32)       # alpha gate: mask[ta=kk]
        lnm[r, 1] = mask[:, T - kk].astype(np.float32)   # beta gate: mask[te=512-kk]
    lnm[NRESC, 0] = 1.0                                  # final rescale: S0 always
    lnm[NRESC, 1] = mask[:, TME].astype(np.float32)      # S1 only if used

    A = np.asarray(inputs["transition"], np.float32)

    def wperm(w):
        return np.ascontiguousarray(np.asarray(w)[perm].T).astype(bf)

    m = {
        "toks": toks.astype(np.int32),
        "masku": mask.T.astype(np.uint8).reshape(1, T * BL),
        "t1h": t1h.astype(bf),
        "tnx": tnx.astype(bf),
        "lnmask": lnm.reshape(1, (NRESC + 1) * 2 * BL),
        "emb": np.asarray(inputs["embedding"]).astype(bf),
        "wih_f": wperm(inputs["w_ih_f"]),
        "wih_b": wperm(inputs["w_ih_b"]),
        "whh_f": wperm(inputs["w_hh_f"]),
        "whh_b": wperm(inputs["w_hh_b"]),
        "bih_f": np.ascontiguousarray(np.asarray(inputs["b_f"])[perm].reshape(8, 128).T).astype(np.float32),
        "bih_b": np.ascontiguousarray(np.asarray(inputs["b_b"])[perm].reshape(8, 128).T).astype(np.float32),
        "bihT_f": np.asarray(inputs["b_f"])[perm].reshape(1, 8 * 128).astype(bf),
        "bihT_b": np.asarray(inputs["b_b"])[perm].reshape(1, 8 * 128).astype(bf),
        "woutT": np.ascontiguousarray(np.asarray(inputs["w_out"]).T.reshape(4, 128, K)).astype(bf),
        "bout": np.asarray(inputs["b_out"]).reshape(K, 1).astype(np.float32),
        "expA": np.exp(A).astype(bf),
        "expAT": np.ascontiguousarray(np.exp(A).T).astype(bf),
        "Abf": A.astype(bf),
        "wstar": np.linalg.solve(np.exp(A), np.ones(K)).reshape(1, K).astype(np.float32),
    }
    return m


def kernel(**inputs):
    from concourse.bass_utils import run_bass_kernel_spmd

    if "prog" not in _cache:
        _cache["prog"] = _build_program()
    nc, names = _cache["prog"]

    in_maps = []
    for kcore in range(NCORES):
        m = _prep_core(inputs, kcore)
        in_maps.append({names[kk]: vv for kk, vv in m.items()})

    res = run_bass_kernel_spmd(nc, in_maps, core_ids=list(range(NCORES)),
                               **_cache.get("run_kwargs", {}))
    out = np.concatenate([r[names["out"]].reshape(BL) for r in res.results])
    _cache["last_results"] = res
    return out.astype(np.float32)


# revision 48
# speedup vs baseline: 2.0868x; 1.0304x over previous
"""BiLSTM-CRF loss kernel for Trainium2 (8 NeuronCores, data-parallel over batch).

v2 design (per core, BL=16 sequences):
  Phase 1 (512 wall-steps): fwd LSTM (t=s) and bwd LSTM (t=511-s) run as two
    interleaved streams so every engine pipelines across streams.
    - Gate order host-permuted to [i,f,o,g] so one Sigmoid op covers i,f,o.
    - Input projection xw is windowed (32 steps) and injected into the gate
      PSUM accumulation via an identity matmul (PE is cheap, DVE is not).
    - Emissions pre-biased once; both streams add their w_out matmul via a
      single joint DVE op per step.
    - Only Sigmoid/Tanh on ACT -> zero activation-table thrash.
    - Gold-score (unary+transition) reductions interleaved into phase-1 slack.
  Phase 2 (~256 wall-steps): CRF log-partition via two interleaved exp-space
    recursions on bulk-exp'd emissions: alpha upward (t=1..256) and beta
    downward (t=510..256), meeting at t*=256. Periodic rescale; the log
    compensation terms are stored and Ln'd once at the end.
"""

import numpy as np

PAD_IDX = 0
VOCAB, K, E, H = 30000, 20, 256, 256
B, T = 128, 512
NCORES = 8
BL = B // NCORES          # 16 sequences per core
WIN = 32                  # proj window (time steps)
NW = T // WIN             # 16 windows
RESC = 8                  # CRF rescale interval (wall steps)
TME = 256                 # CRF meeting point: logZ = ln(sum D_TME * B_TME)
NRESC = 31                # rescales at k=8,16,...,248

_cache = {}


def _build_program():
    from contextlib import ExitStack
    import concourse.bass as bass
    import concourse.bacc as bacc
    import concourse.tile as tile
    from concourse import mybir
    from concourse.masks import make_identity

    f32 = mybir.dt.float32
    bf16 = mybir.dt.bfloat16
    i32 = mybir.dt.int32
    u8 = mybir.dt.uint8
    AF = mybir.ActivationFunctionType
    OP = mybir.AluOpType

    nc = bacc.Bacc(None, target_bir_lowering=False, debug=False)
    names = {}

    with ExitStack() as ctx:
        tc = ctx.enter_context(tile.TileContext(nc))
        dram = ctx.enter_context(tc.tile_pool(name="dram", bufs=1, space="DRAM"))

        def din(key, shape, dt=f32):
            t = dram.tile(shape, dt, kind="ExternalInput", name=key)
            names[key] = t.tensor.name
            return t

        emb = din("emb", [VOCAB, E], bf16)
        toks = din("toks", [T * BL, 1], i32)            # window-major token ids
        masku = din("masku", [1, T * BL], u8)           # col = t*16+b
        invmu = din("invmu", [1, T * BL], u8)           # 1 - mask
        t1h = din("t1h", [K, T * BL], bf16)             # one-hot(tag)*mask
        tnx = din("tnx", [K, T * BL], bf16)             # shifted one-hot*mask
        lnmask = din("lnmask", [1, (NRESC + 1) * 2 * BL])  # mask at rescale times
        wih = {d: din(f"wih_{d}", [E, 4 * H], bf16) for d in "fb"}
        whh = {d: din(f"whh_{d}", [E, 4 * H], bf16) for d in "fb"}
        bih = {d: din(f"bih_{d}", [128, 8]) for d in "fb"}
        bihT = {d: din(f"bihT_{d}", [1, 8 * 128], bf16) for d in "fb"}
        woutT = din("woutT", [4, 128, K], bf16)         # chunks: Fk0,Fk1,Bk0,Bk1
        bout = din("bout", [K, 1])
        expA = din("expA", [K, K], bf16)                # exp(transition)
        expAT = din("expAT", [K, K], bf16)              # exp(transition).T
        Abf = din("Abf", [K, K], bf16)                  # transition (bf16)
        wstar = din("wstar", [1, K])                    # 1^T (expA^T)^{-1}
        out_loss = dram.tile([1, BL], f32, kind="ExternalOutput")
        names["out"] = out_loss.tensor.name

        sg = ctx.enter_context(tc.tile_pool(name="sg", bufs=1))       # singles
        tmp = ctx.enter_context(tc.tile_pool(name="tmp", bufs=4))     # step temps
        gat = ctx.enter_context(tc.tile_pool(name="gat", bufs=4))     # gather tiles
        fin = ctx.enter_context(tc.tile_pool(name="fin", bufs=3))     # finalize
        ps_g = ctx.enter_context(tc.tile_pool(name="ps_g", bufs=2, space="PSUM"))
        ps_w = ctx.enter_context(tc.tile_pool(name="ps_w", bufs=1, space="PSUM"))
        ps_t = ctx.enter_context(tc.tile_pool(name="ps_t", bufs=1, space="PSUM"))
        ps_s = ctx.enter_context(tc.tile_pool(name="ps_s", bufs=2, space="PSUM"))

        # ---- resident SBUF tensors ----
        s_wih = {d: sg.tile([128, 2, 4 * H], bf16, tag=f"wih{d}", name=f"wih{d}") for d in "fb"}
        s_whh = {d: sg.tile([128, 2, 4 * H], bf16, tag=f"whh{d}", name=f"whh{d}") for d in "fb"}
        for d in "fb":
            nc.sync.dma_start(out=s_wih[d][:], in_=wih[d][:].rearrange("(k p) m -> p k m", p=128))
            nc.sync.dma_start(out=s_whh[d][:], in_=whh[d][:].rearrange("(k p) m -> p k m", p=128))
        s_bih = {d: sg.tile([128, 8], f32, tag=f"bih{d}", name=f"bih{d}") for d in "fb"}
        s_bihT = {d: sg.tile([1, 8, 128], bf16, tag=f"bihT{d}", name=f"bihT{d}") for d in "fb"}
        for d in "fb":
            nc.sync.dma_start(out=s_bih[d][:], in_=bih[d][:])
            nc.sync.dma_start(out=s_bihT[d][:].rearrange("o m p -> o (m p)"), in_=bihT[d][:])
        onesb = sg.tile([1, 512], bf16, tag="onesb")
        nc.vector.memset(onesb[:], 1.0)
        s_wout = sg.tile([128, 4, K], bf16, tag="wout")
        nc.sync.dma_start(out=s_wout[:], in_=woutT[:].rearrange("c p k -> p c k"))
        s_bout = sg.tile([K, 1], f32, tag="bout")
        nc.sync.dma_start(out=s_bout[:], in_=bout[:])
        s_eA = sg.tile([K, K], bf16, tag="eA")
        nc.sync.dma_start(out=s_eA[:], in_=expA[:])
        s_eAT = sg.tile([K, K], bf16, tag="eAT")
        nc.sync.dma_start(out=s_eAT[:], in_=expAT[:])
        s_A = sg.tile([K, K], bf16, tag="A")
        nc.sync.dma_start(out=s_A[:], in_=Abf[:])
        s_ws = sg.tile([1, K], f32, tag="ws")
        nc.sync.dma_start(out=s_ws[:], in_=wstar[:])
        s_t1h = sg.tile([K, T * BL], bf16, tag="t1h")
        nc.sync.dma_start(out=s_t1h[:], in_=t1h[:])
        s_tnx = sg.tile([K, T * BL], bf16, tag="tnx")
        nc.sync.dma_start(out=s_tnx[:], in_=tnx[:])
        s_lnm = sg.tile([1, (NRESC + 1) * 2 * BL], f32, tag="lnm")
        nc.sync.dma_start(out=s_lnm[:], in_=lnmask[:])

        ones = sg.tile([128, K], f32, tag="ones")
        nc.vector.memset(ones[:], 1.0)
        ident = sg.tile([128, 128], bf16, tag="ident")
        make_identity(nc, ident[:])

        # mask replica: (128, T, BL), col = t*16+b, broadcast across partitions
        maskrep = sg.tile([128, T, BL], u8, tag="maskrep")
        nc.sync.dma_start(
            out=maskrep[:],
            in_=bass.AP(tensor=masku.tensor, offset=masku[:].offset,
                        ap=[[0, 128], [BL, T], [1, BL]]),
        )
        invrep = sg.tile([128, T, BL], u8, tag="invrep")
        nc.sync.dma_start(
            out=invrep[:],
            in_=bass.AP(tensor=invmu.tensor, offset=invmu[:].offset,
                        ap=[[0, 128], [BL, T], [1, BL]]),
        )
        neg50 = sg.tile([128, 1], bf16, tag="neg50")
        nc.vector.memset(neg50[:], -50.0)

        # token indices for gathers (128 per column)
        NT128 = T * BL // 128
        idxall = sg.tile([128, NT128], i32, tag="idxall")
        nc.sync.dma_start(out=idxall[:],
                          in_=bass.AP(tensor=toks.tensor, offset=toks[:].offset,
                                      ap=[[1, 128], [128, NT128]]))

        # emissions (f32) and their exp; pre-bias emit with b_out
        emit = sg.tile([K, T, BL], f32, tag="emit")
        emf0 = emit[:].rearrange("k t b -> k (t b)")
        nc.vector.memset(emf0, 0.0)
        nc.vector.tensor_scalar_add(emf0, emf0, s_bout[:, 0:1])
        expE = emit  # exp taken in place after phase 1 (unary reads are done by then)

        # per-direction rotating transposed-input windows and projection windows
        xtp = ctx.enter_context(tc.tile_pool(name="xtp", bufs=2))
        winp = ctx.enter_context(tc.tile_pool(name="winp", bufs=2))

        # LSTM states, layout [128, dir(2), k(2), BL]
        st_h = sg.tile([128, 2, 2, BL], bf16, tag="st_h")
        st_c = sg.tile([128, 2, 2, BL], f32, tag="st_c")
        nc.vector.memset(st_h[:], 0.0)
        nc.vector.memset(st_c[:], 0.0)

        # CRF state S[:, 0]=alpha D, S[:, 1]=beta B; ln-compensation buffer
        S = sg.tile([K, 2, BL], f32, tag="S")
        lnbuf = sg.tile([1, NRESC + 1, 2, BL], f32, tag="lnbuf")
        Uacc = sg.tile([K, BL], f32, tag="Uacc")
        nc.vector.memset(Uacc[:], 0.0)
        TRacc = sg.tile([K, BL], f32, tag="TRacc")
        nc.vector.memset(TRacc[:], 0.0)

        def gather_window(w, d):
            """Gather+transpose window w into a rotating per-direction xT tile."""
            xT = xtp.tile([128, 2, 512], bf16, tag=f"xT{d}", name=f"xT{d}")
            for g in range(4):
                j = w * 4 + g
                xg = gat.tile([128, E], bf16, tag="xg")
                nc.gpsimd.indirect_dma_start(
                    out=xg[:], out_offset=None, in_=emb[:],
                    in_offset=bass.IndirectOffsetOnAxis(ap=idxall[:, j:j + 1], axis=0),
                )
                for k in range(2):
                    pst = ps_t.tile([128, 128], bf16, tag="pst")
                    nc.tensor.transpose(out=pst[:], in_=xg[:, k * 128:(k + 1) * 128],
                                        identity=ident[:])
                    if (g + k) % 2 == 0:
                        nc.vector.tensor_copy(xT[:, k, g * 128:(g + 1) * 128], pst[:])
                    else:
                        nc.scalar.activation(xT[:, k, g * 128:(g + 1) * 128], pst[:], AF.Copy)
            return xT

        def make_window(xT, d, w):
            """Project a gathered window for direction d -> win (128, 8, BL, WIN)
            bf16, bias folded via an extra matmul. psw column order: (b, t).
            For the bwd stream, clamp the i-gate pre-activation to -50 at padded
            positions so the cell state stays exactly 0 through the pad suffix."""
            win = winp.tile([128, 8, BL, WIN], bf16, tag=f"win{d}", name=f"win{d}")
            for m in range(8):
                psw = ps_w.tile([128, 512], f32, tag="psw")
                for k in range(2):
                    nc.tensor.matmul(psw[:], lhsT=s_wih[d][:, k, m * 128:(m + 1) * 128],
                                     rhs=xT[:, k, :], start=(k == 0), stop=False)
                nc.tensor.matmul(psw[:], lhsT=s_bihT[d][:, m, :], rhs=onesb[:],
                                 start=False, stop=True)
                wv = win[:, m].rearrange("p b t -> p (b t)")
                if m % 2 == 0:
                    nc.scalar.activation(wv, psw[:], AF.Copy)
                else:
                    nc.vector.tensor_copy(wv, psw[:])
            if d == "b":
                pred = bass.AP(tensor=invrep.tensor,
                               offset=invrep[0:128, w * WIN, 0:BL].offset,
                               ap=[invrep[:].ap[0], [0, 2], [1, BL], [BL, WIN]])
                data = bass.AP(tensor=neg50.tensor, offset=neg50[:].offset,
                               ap=[neg50[:].ap[0], [0, 2], [0, BL], [0, WIN]])
                nc.vector.copy_predicated(win[:, 0:2], pred, data)
            return win

        # mask AP helpers --------------------------------------------------
        def mask_joint(tlo, thi, parts, reps):
            """(parts, 2, reps, BL) AP over maskrep: dir0 at t=tlo, dir1 at t=thi."""
            base = maskrep[0:parts, tlo, 0:BL]
            return bass.AP(tensor=base.tensor, offset=base.offset,
                           ap=[base.ap[0], [(thi - tlo) * BL, 2], [0, reps], [1, BL]])

        def mask_one(t, parts, reps):
            base = maskrep[0:parts, t, 0:BL]
            if reps == 1:
                return base
            return bass.AP(tensor=base.tensor, offset=base.offset,
                           ap=[base.ap[0], [0, reps], [1, BL]])

        # -------- gold-score chunk work (interleaved into phase 1) --------
        NCH = 16
        CW = T * BL // NCH          # 512 cols per chunk

        def trans_chunk(ci):
            """TRacc += reduce_t(A[tag, :] * shifted-onehot) for chunk ci."""
            psa = ps_w.tile([K, CW], f32, tag="psw", name="psa")
            nc.tensor.matmul(psa[:], lhsT=s_A[:], rhs=s_t1h[:, ci * CW:(ci + 1) * CW],
                             start=True, stop=True)
            um = fin.tile([K, CW], f32, tag="um")
            nc.vector.tensor_tensor(um[:], psa[:], s_tnx[:, ci * CW:(ci + 1) * CW], op=OP.mult)
            ur = fin.tile([K, BL], f32, tag="ur")
            umr = bass.AP(tensor=um.tensor, offset=um[:].offset,
                          ap=[um[:].ap[0], [1, BL], [BL, CW // BL]])
            nc.vector.tensor_reduce(ur[:], umr, axis=mybir.AxisListType.X, op=OP.add)
            nc.vector.tensor_tensor(TRacc[:], TRacc[:], ur[:], op=OP.add)

        def unary_chunk(ci):
            """Uacc += reduce_t(onehot * emit) for chunk ci (emit must be complete)."""
            um = fin.tile([K, CW], f32, tag="um")
            ef = emit[:].rearrange("k t b -> k (t b)")
            nc.gpsimd.tensor_tensor(um[:], s_t1h[:, ci * CW:(ci + 1) * CW],
                                    ef[:, ci * CW:(ci + 1) * CW], op=OP.mult)
            ur = fin.tile([K, BL], f32, tag="ur")
            umr = bass.AP(tensor=um.tensor, offset=um[:].offset,
                          ap=[um[:].ap[0], [1, BL], [BL, CW // BL]])
            nc.vector.tensor_reduce(ur[:], umr, axis=mybir.AxisListType.X, op=OP.add)
            nc.vector.tensor_tensor(Uacc[:], Uacc[:], ur[:], op=OP.add)

        # emit chunk completion wall-step: chunk ci covers t in [ci*32,(ci+1)*32)
        unary_sched = {}
        for ci in range(NCH):
            t0c, t1c = ci * (T // NCH), (ci + 1) * (T // NCH) - 1
            done = max(t1c, T - 1 - t0c) + 1   # +1: emit adds land one step late
            unary_sched.setdefault(min(done, T), []).append(ci)

        # ---------------- phase 1: interleaved fwd/bwd LSTM ----------------
        # prefetch first windows
        win_cur = {"f": make_window(gather_window(0, "f"), "f", 0),
                   "b": make_window(gather_window(15, "b"), "b", 15)}
        win_nxt = {}
        xt_nxt = {}

        warm = ps_s.tile([1, 1], f32, tag="pssm")
        nc.tensor.matmul(warm[:], lhsT=ident[:, 0:1], rhs=ident[:, 0:1], start=True, stop=True)

        def do_pse(s):
            """Emission matmuls + delayed joint emit add for step s."""
            tf, tb = s, T - 1 - s
            tlo, thi = (tf, tb) if tf < tb else (tb, tf)
            jf = 0 if tf < tb else 1
            pse = ps_s.tile([K, 2, BL], f32, tag="pssm", name="pse")
            for k in range(2):
                nc.tensor.matmul(pse[:, jf], lhsT=s_wout[:, k, :], rhs=st_h[:, 0, k, :],
                                 start=(k == 0), stop=(k == 1))
            for k in range(2):
                nc.tensor.matmul(pse[:, 1 - jf], lhsT=s_wout[:, 2 + k, :], rhs=st_h[:, 1, k, :],
                                 start=(k == 0), stop=(k == 1))
            eap = bass.AP(tensor=emit.tensor, offset=emit[:, tlo, :].offset,
                          ap=[emit[:].ap[0], [(thi - tlo) * BL, 2], [1, BL]])
            nc.vector.tensor_tensor(eap, eap, pse[:], op=OP.add)

        def burst(d, di, s):
            """Recurrence + window-injection matmuls for one stream."""
            toff = s % WIN
            tof = toff if d == "f" else WIN - 1 - toff
            psg = ps_g.tile([128, 8, BL], f32, tag=f"psg{d}", name=f"psg{d}")
            for m in range(8):
                for k in range(2):
                    nc.tensor.matmul(psg[:, m], lhsT=s_whh[d][:, k, m * 128:(m + 1) * 128],
                                     rhs=st_h[:, di, k, :], start=(m == 0 and k == 0), stop=False)
            wslice = bass.AP(tensor=win_cur[d].tensor,
                             offset=win_cur[d][:, :, :, tof].offset,
                             ap=[win_cur[d][:].ap[0], [BL * WIN, 8], [WIN, BL]])
            nc.tensor.matmul(psg[:], lhsT=ident[:], rhs=wslice, start=False, stop=True)
            return psg

        for s in range(T):
            blk, toff = divmod(s, WIN)

            # ---- stream f: matmuls, activations, cell update (all in place) ----
            psgf = burst("f", 0, s)
            gf = tmp.tile([128, 8, BL], f32, tag="gatf", name="gatf")
            nc.scalar.activation(gf[:], psgf[:], AF.Sigmoid)
            nc.gpsimd.tensor_scalar(gf[:, 6:8, :], gf[:, 6:8, :], 2.0, -1.0,
                                    op0=OP.mult, op1=OP.add)
            igf = tmp.tile([128, 2, BL], f32, tag="igf", name="igf")
            nc.gpsimd.tensor_tensor(igf[:], gf[:, 0:2, :], gf[:, 6:8, :], op=OP.mult)
            nc.vector.tensor_tensor(st_c[:, 0], gf[:, 2:4, :], st_c[:, 0], op=OP.mult)
            nc.vector.tensor_tensor(st_c[:, 0], st_c[:, 0], igf[:], op=OP.add)

            # ---- stream b: matmuls + emissions of step s-1 on PE ----
            psgb = burst("b", 1, s)
            if s > 0:
                do_pse(s - 1)
            gb = tmp.tile([128, 8, BL], f32, tag="gatb", name="gatb")
            nc.scalar.activation(gb[:], psgb[:], AF.Sigmoid)

            # ---- stream f tail: tanh(c), h ----
            thf = tmp.tile([128, 2, BL], f32, tag="thf", name="thf")
            nc.scalar.activation(thf[:], st_c[:, 0], AF.Tanh)
            nc.gpsimd.tensor_scalar(gb[:, 6:8, :], gb[:, 6:8, :], 2.0, -1.0,
                                    op0=OP.mult, op1=OP.add)
            igb = tmp.tile([128, 2, BL], f32, tag="igb", name="igb")
            nc.gpsimd.tensor_tensor(igb[:], gb[:, 0:2, :], gb[:, 6:8, :], op=OP.mult)
            nc.vector.tensor_tensor(st_c[:, 1], gb[:, 2:4, :], st_c[:, 1], op=OP.mult)
            nc.vector.tensor_tensor(st_h[:, 0], gf[:, 4:6, :], thf[:], op=OP.mult)
            nc.vector.tensor_tensor(st_c[:, 1], st_c[:, 1], igb[:], op=OP.add)
            thb = tmp.tile([128, 2, BL], f32, tag="thb", name="thb")
            nc.scalar.activation(thb[:], st_c[:, 1], AF.Tanh)
            nc.vector.tensor_tensor(st_h[:, 1], gb[:, 4:6, :], thb[:], op=OP.mult)

            # window prefetch/projection for the next block, spread over the block
            if blk < NW - 1:
                if toff == 4:
                    xt_nxt["f"] = gather_window(blk + 1, "f")
                if toff == 12:
                    xt_nxt["b"] = gather_window(NW - 2 - blk, "b")
                if toff == 20:
                    win_nxt["f"] = make_window(xt_nxt["f"], "f", blk + 1)
                if toff == 26:
                    win_nxt["b"] = make_window(xt_nxt["b"], "b", NW - 2 - blk)
                if toff == WIN - 1:
                    win_cur = dict(win_nxt)

            # interleave gold-score chunks
            if s % 32 == 16 and s // 32 < NCH // 2:
                ci = s // 32
                trans_chunk(2 * ci)
                trans_chunk(2 * ci + 1)
            for ci in unary_sched.get(s, []):
                unary_chunk(ci)

        do_pse(T - 1)
        for ci in unary_sched.get(T, []):
            unary_chunk(ci)

        # ---------------- phase 1.5: bulk exp ----------------
        ef = emit[:].rearrange("k t b -> k (t b)")
        xf = expE[:].rearrange("k t b -> k (t b)")
        for q in range(4):
            sl = slice(q * T * BL // 4, (q + 1) * T * BL // 4)
            nc.scalar.activation(xf[:, sl], ef[:, sl], AF.Exp)

        # ---------------- phase 2: CRF alpha/beta meeting at TME ----------------
        # Alpha runs in Q-space (Q_t = expA^T @ D_t) so both streams share the
        # "multiply by expE, then matmul" shape:
        #   alpha (dir 0): S0 <- masked_{m[ta]}  (expA^T @ (S0 * expE[ta])),  ta = 0..TME-1
        #   beta  (dir 1): S1 <- masked_{m[te]}  (expA   @ (S1 * expE[te])),  te = 511..TME+1
        # final: logZ = ln sum_i (expE[TME] * Q_{TME-1} * B_TME)[i] + ln-comp terms
        nc.vector.memset(S[:], 1.0)

        nrs = 0
        for kk in range(TME):
            ta = kk                # alpha uses expE[ta], mask[ta]
            te = T - kk            # beta uses expE[te], mask[te]; skip kk=0 (te=512)
            bp = tmp.tile([K, 2, BL], bf16, tag="bp", name="bp")
            psj = ps_s.tile([K, 2, BL], f32, tag="pssm", name="psj")
            if kk == 0:
                nc.vector.tensor_tensor(bp[:, 0], S[:, 0], expE[:, ta, :], op=OP.mult)
                nc.tensor.matmul(psj[:, 0], lhsT=s_eA[:], rhs=bp[:, 0], start=True, stop=True)
                nc.vector.copy_predicated(S[:, 0], mask_one(ta, K, 1), psj[:, 0])
                continue
            # joint: dir0 alpha reads expE[ta], dir1 beta reads expE[te]
            xap = bass.AP(tensor=expE.tensor, offset=expE[:, ta, :].offset,
                          ap=[expE[:].ap[0], [(te - ta) * BL, 2], [1, BL]])
            nc.vector.tensor_tensor(bp[:], S[:], xap, op=OP.mult)
            nc.tensor.matmul(psj[:, 0], lhsT=s_eA[:], rhs=bp[:, 0], start=True, stop=True)
            nc.tensor.matmul(psj[:, 1], lhsT=s_eAT[:], rhs=bp[:, 1], start=True, stop=True)
            nc.vector.copy_predicated(S[:], mask_joint(ta, te, K, 1), psj[:])

            if kk % RESC == 0 and kk >= 8 and kk <= 248:
                # rescale both streams; ln terms batched at the end
                pss = ps_s.tile([1, 2 * BL], f32, tag="pssm", name="pss")
                nc.tensor.matmul(pss[:], lhsT=ones[0:K, 0:1],
                                 rhs=S[:].rearrange("k d b -> k (d b)"), start=True, stop=True)
                nc.vector.tensor_copy(lnbuf[:, nrs].rearrange("o d b -> o (d b)"), pss[:])
                rr = tmp.tile([1, 2 * BL], f32, tag="rr")
                nc.vector.reciprocal(rr[:], pss[:])
                psr = ps_s.tile([K, 2 * BL], f32, tag="pssm", name="psr")
                nc.tensor.matmul(psr[:], lhsT=ones[0:1, 0:K], rhs=rr[:], start=True, stop=True)
                sc = tmp.tile([K, 2, BL], f32, tag="sc")
                nc.vector.tensor_tensor(sc[:].rearrange("k d b -> k (d b)"),
                                        S[:].rearrange("k d b -> k (d b)"), psr[:], op=OP.mult)
                nc.vector.copy_predicated(S[:], mask_joint(ta, te, K, 1), sc[:])
                nrs += 1
        assert nrs == NRESC, nrs

        # final compensated rescale (bounds the final Ln input); plain scaling,
        # the lnmask row gates S1's compensation to columns that use it
        pss = ps_s.tile([1, 2 * BL], f32, tag="pssm", name="pssf")
        nc.tensor.matmul(pss[:], lhsT=ones[0:K, 0:1],
                         rhs=S[:].rearrange("k d b -> k (d b)"), start=True, stop=True)
        nc.vector.tensor_copy(lnbuf[:, NRESC].rearrange("o d b -> o (d b)"), pss[:])
        rr = tmp.tile([1, 2 * BL], f32, tag="rr")
        nc.vector.reciprocal(rr[:], pss[:])
        psr = ps_s.tile([K, 2 * BL], f32, tag="pssm", name="psrf")
        nc.tensor.matmul(psr[:], lhsT=ones[0:1, 0:K], rhs=rr[:], start=True, stop=True)
        nc.vector.tensor_tensor(S[:].rearrange("k d b -> k (d b)"),
                                S[:].rearrange("k d b -> k (d b)"), psr[:], op=OP.mult)

        # ---------------- finalize ----------------
        # logZ = ln(sum_i D[i]*B[i]) + sum(masked ln rescale terms)
        lns = fin.tile([1, (NRESC + 1) * 2 * BL], f32, tag="lns")
        nc.scalar.activation(lns[:], lnbuf[:].rearrange("o r d b -> o (r d b)"), AF.Ln)
        nc.vector.tensor_tensor(lns[:], lns[:], s_lnm[:], op=OP.mult)
        lnred = fin.tile([1, BL], f32, tag="lnred")
        lnsr = bass.AP(tensor=lns.tensor, offset=lns[:].offset,
                       ap=[lns[:].ap[0], [1, BL], [BL, (NRESC + 1) * 2]])
        nc.vector.tensor_reduce(lnred[:], lnsr, axis=mybir.AxisListType.X, op=OP.add)

        # combine vector V: active cols (len > TME) use expE[TME]*beta,
        # frozen cols (len <= TME) use w* (recovers sum_i D_i from Q-space)
        psR = ps_s.tile([K, BL], f32, tag="pssm", name="psR")
        nc.tensor.matmul(psR[:], lhsT=s_ws[:], rhs=ones[0:1, 0:BL], start=True, stop=True)
        V = fin.tile([K, BL], f32, tag="V")
        nc.vector.tensor_copy(V[:], psR[:])
        EV = fin.tile([K, BL], f32, tag="EV")
        nc.vector.tensor_tensor(EV[:], expE[:, TME, :], S[:, 1], op=OP.mult)
        nc.vector.copy_predicated(V[:], mask_one(TME, K, 1), EV[:])
        zt = fin.tile([K, BL], f32, tag="zt")
        nc.vector.tensor_tensor(zt[:], S[:, 0], V[:], op=OP.mult)
        psz = ps_s.tile([1, BL], f32, tag="pssm", name="psz")
        nc.tensor.matmul(psz[:], lhsT=ones[0:K, 0:1], rhs=zt[:], start=True, stop=True)
        logZ = fin.tile([1, BL], f32, tag="logZ")
        nc.scalar.activation(logZ[:], psz[:], AF.Ln)
        nc.vector.tensor_tensor(logZ[:], logZ[:], lnred[:], op=OP.add)

        # gold score = colsum(Uacc) + colsum(TRacc)
        nc.vector.tensor_tensor(Uacc[:], Uacc[:], TRacc[:], op=OP.add)
        psu = ps_s.tile([1, BL], f32, tag="pssm", name="psu")
        nc.tensor.matmul(psu[:], lhsT=ones[0:K, 0:1], rhs=Uacc[:], start=True, stop=True)

        res = fin.tile([1, BL], f32, tag="res")
        nc.vector.tensor_tensor(res[:], logZ[:], psu[:], op=OP.subtract)
        nc.sync.dma_start(out=out_loss[:], in_=res[:])

    nc.compile()
    return nc, names


def _prep_core(inputs, kcore):
    """Per-core host-side input prep (index plumbing + layout shuffles)."""
    import ml_dtypes
    bf = ml_dtypes.bfloat16
    s = slice(kcore * BL, (kcore + 1) * BL)
    sent = np.asarray(inputs["sentences"][s])          # (16, 512) i32
    tags = np.asarray(inputs["tags"][s])               # (16, 512) i32
    mask = (sent != PAD_IDX)                           # (16, 512)

    # gate-row permutation [i,f,g,o] -> [i,f,o,g]; g rows scaled x2 so that
    # tanh(g) can be computed as 2*sigmoid(2g)-1 with a single Sigmoid op
    perm = np.concatenate([np.arange(0, 2 * H), np.arange(3 * H, 4 * H),
                           np.arange(2 * H, 3 * H)])
    gsc = np.ones((4 * H, 1), np.float32)
    gsc[3 * H:] = 2.0

    toks = sent.reshape(BL, NW, WIN).transpose(1, 0, 2).reshape(T * BL, 1)
    oh = (tags[:, :, None] == np.arange(K)[None, None, :])
    t1h = (oh & mask[:, :, None]).transpose(2, 1, 0).reshape(K, T * BL)
    tnxm = np.zeros((BL, T, K), np.float32)
    tnxm[:, :-1, :] = (oh[:, 1:, :] & mask[:, 1:, None]).astype(np.float32)
    tnx = tnxm.transpose(2, 1, 0).reshape(K, T * BL)

    # lnmask: mask value at the rescale checkpoints, layout (r, dir, b)
    lnm = np.zeros((NRESC + 1, 2, BL), np.float32)
    for r in range(NRESC):
        kk = 8 * (r + 1)
        lnm[r, 0] = mask[:, kk].astype(np.float32)       # alpha gate: mask[ta=kk]
        lnm[r, 1] = mask[:, T - kk].astype(np.float32)   # beta gate: mask[te=512-kk]
    lnm[NRESC, 0] = 1.0                                  # final rescale: S0 always
    lnm[NRESC, 1] = mask[:, TME].astype(np.float32)      # S1 only if used

    A = np.asarray(inputs["transition"], np.float32)

    def wperm(w):
        return np.ascontiguousarray((np.asarray(w)[perm] * gsc).T).astype(bf)

    def bperm(b):
        return np.asarray(b)[perm] * gsc[:, 0]

    m = {
        "toks": toks.astype(np.int32),
        "masku": mask.T.astype(np.uint8).reshape(1, T * BL),
        "invmu": (~mask).T.astype(np.uint8).reshape(1, T * BL),
        "t1h": t1h.astype(bf),
        "tnx": tnx.astype(bf),
        "lnmask": lnm.reshape(1, (NRESC + 1) * 2 * BL),
        "emb": np.asarray(inputs["embedding"]).astype(bf),
        "wih_f": wperm(inputs["w_ih_f"]),
        "wih_b": wperm(inputs["w_ih_b"]),
        "whh_f": wperm(inputs["w_hh_f"]),
        "whh_b": wperm(inputs["w_hh_b"]),
        "bih_f": np.ascontiguousarray(bperm(inputs["b_f"]).reshape(8, 128).T).astype(np.float32),
        "bih_b": np.ascontiguousarray(bperm(inputs["b_b"]).reshape(8, 128).T).astype(np.float32),
        "bihT_f": bperm(inputs["b_f"]).reshape(1, 8 * 128).astype(bf),
        "bihT_b": bperm(inputs["b_b"]).reshape(1, 8 * 128).astype(bf),
        "woutT": np.ascontiguousarray(np.asarray(inputs["w_out"]).T.reshape(4, 128, K)).astype(bf),
        "bout": np.asarray(inputs["b_out"]).reshape(K, 1).astype(np.float32),
        "expA": np.exp(A).astype(bf),
        "expAT": np.ascontiguousarray(np.exp(A).T).astype(bf),
        "Abf": A.astype(bf),
        "wstar": np.linalg.solve(np.exp(A), np.ones(K)).reshape(1, K).astype(np.float32),
    }
    return m


def kernel(**inputs):
    from concourse.bass_utils import run_bass_kernel_spmd

    if "prog" not in _cache:
        _cache["prog"] = _build_program()
    nc, names = _cache["prog"]

    in_maps = []
    for kcore in range(NCORES):
        m = _prep_core(inputs, kcore)
        in_maps.append({names[kk]: vv for kk, vv in m.items()})

    res = run_bass_kernel_spmd(nc, in_maps, core_ids=list(range(NCORES)),
                               **_cache.get("run_kwargs", {}))
    out = np.concatenate([r[names["out"]].reshape(BL) for r in res.results])
    _cache["last_results"] = res
    return out.astype(np.float32)


# revision 54
# speedup vs baseline: 2.5993x; 1.2456x over previous
"""BiLSTM-CRF loss kernel for Trainium2 (8 NeuronCores, data-parallel over batch).

v2 design (per core, BL=16 sequences):
  Phase 1 (512 wall-steps): fwd LSTM (t=s) and bwd LSTM (t=511-s) run as two
    interleaved streams so every engine pipelines across streams.
    - Gate order host-permuted to [i,f,o,g] so one Sigmoid op covers i,f,o.
    - Input projection xw is windowed (32 steps) and injected into the gate
      PSUM accumulation via an identity matmul (PE is cheap, DVE is not).
    - Emissions pre-biased once; both streams add their w_out matmul via a
      single joint DVE op per step.
    - Only Sigmoid/Tanh on ACT -> zero activation-table thrash.
    - Gold-score (unary+transition) reductions interleaved into phase-1 slack.
  Phase 2 (~256 wall-steps): CRF log-partition via two interleaved exp-space
    recursions on bulk-exp'd emissions: alpha upward (t=1..256) and beta
    downward (t=510..256), meeting at t*=256. Periodic rescale; the log
    compensation terms are stored and Ln'd once at the end.
"""

import numpy as np

PAD_IDX = 0
VOCAB, K, E, H = 30000, 20, 256, 256
B, T = 128, 512
NCORES = 8
BL = B // NCORES          # 16 sequences per core
WIN = 32                  # proj window (time steps)
NW = T // WIN             # 16 windows
RESC = 8                  # CRF rescale interval (wall steps)
TME = 256                 # CRF meeting point: logZ = ln(sum D_TME * B_TME)
NRESC = 31                # rescales at k=8,16,...,248

_cache = {}


def _build_program():
    from contextlib import ExitStack
    import concourse.bass as bass
    import concourse.bacc as bacc
    import concourse.tile as tile
    from concourse import mybir
    from concourse.masks import make_identity

    f32 = mybir.dt.float32
    bf16 = mybir.dt.bfloat16
    i32 = mybir.dt.int32
    u8 = mybir.dt.uint8
    AF = mybir.ActivationFunctionType
    OP = mybir.AluOpType

    nc = bacc.Bacc(None, target_bir_lowering=False, debug=False)
    names = {}

    with ExitStack() as ctx:
        tc = ctx.enter_context(tile.TileContext(nc))
        dram = ctx.enter_context(tc.tile_pool(name="dram", bufs=1, space="DRAM"))

        def din(key, shape, dt=f32):
            t = dram.tile(shape, dt, kind="ExternalInput", name=key)
            names[key] = t.tensor.name
            return t

        emb = din("emb", [VOCAB, E], bf16)
        toks = din("toks", [T * BL, 1], i32)            # window-major token ids
        masku = din("masku", [1, T * BL], u8)           # col = t*16+b
        invmu = din("invmu", [1, T * BL], u8)           # 1 - mask
        t1h = din("t1h", [K, T * BL], bf16)             # one-hot(tag)*mask
        tnx = din("tnx", [K, T * BL], bf16)             # shifted one-hot*mask
        lnmask = din("lnmask", [1, (NRESC + 1) * 2 * BL])  # mask at rescale times
        wih = {d: din(f"wih_{d}", [E, 4 * H], bf16) for d in "fb"}
        whh = {d: din(f"whh_{d}", [E, 4 * H], bf16) for d in "fb"}
        bih = {d: din(f"bih_{d}", [128, 8]) for d in "fb"}
        bihT = {d: din(f"bihT_{d}", [1, 8 * 128], bf16) for d in "fb"}
        woutT = din("woutT", [4, 128, K], bf16)         # chunks: Fk0,Fk1,Bk0,Bk1
        bout = din("bout", [K, 1])
        expA = din("expA", [K, K], bf16)                # exp(transition)
        expAT = din("expAT", [K, K], bf16)              # exp(transition).T
        Abf = din("Abf", [K, K], bf16)                  # transition (bf16)
        wstar = din("wstar", [1, K])                    # 1^T (expA^T)^{-1}
        out_loss = dram.tile([1, BL], f32, kind="ExternalOutput")
        names["out"] = out_loss.tensor.name

        sg = ctx.enter_context(tc.tile_pool(name="sg", bufs=1))       # singles
        tmp = ctx.enter_context(tc.tile_pool(name="tmp", bufs=4))     # step temps
        gat = ctx.enter_context(tc.tile_pool(name="gat", bufs=4))     # gather tiles
        fin = ctx.enter_context(tc.tile_pool(name="fin", bufs=3))     # finalize
        ps_g = ctx.enter_context(tc.tile_pool(name="ps_g", bufs=2, space="PSUM"))
        ps_w = ctx.enter_context(tc.tile_pool(name="ps_w", bufs=1, space="PSUM"))
        ps_t = ctx.enter_context(tc.tile_pool(name="ps_t", bufs=1, space="PSUM"))
        ps_s = ctx.enter_context(tc.tile_pool(name="ps_s", bufs=2, space="PSUM"))

        # ---- resident SBUF tensors ----
        s_wih = {d: sg.tile([128, 2, 4 * H], bf16, tag=f"wih{d}", name=f"wih{d}") for d in "fb"}
        s_whh = {d: sg.tile([128, 2, 4 * H], bf16, tag=f"whh{d}", name=f"whh{d}") for d in "fb"}
        for d in "fb":
            nc.sync.dma_start(out=s_wih[d][:], in_=wih[d][:].rearrange("(k p) m -> p k m", p=128))
            nc.sync.dma_start(out=s_whh[d][:], in_=whh[d][:].rearrange("(k p) m -> p k m", p=128))
        s_bih = {d: sg.tile([128, 8], f32, tag=f"bih{d}", name=f"bih{d}") for d in "fb"}
        s_bihT = {d: sg.tile([1, 8, 128], bf16, tag=f"bihT{d}", name=f"bihT{d}") for d in "fb"}
        for d in "fb":
            nc.sync.dma_start(out=s_bih[d][:], in_=bih[d][:])
            nc.sync.dma_start(out=s_bihT[d][:].rearrange("o m p -> o (m p)"), in_=bihT[d][:])
        onesb = sg.tile([1, 512], bf16, tag="onesb")
        nc.vector.memset(onesb[:], 1.0)
        s_wout = sg.tile([128, 4, K], bf16, tag="wout")
        nc.sync.dma_start(out=s_wout[:], in_=woutT[:].rearrange("c p k -> p c k"))
        s_bout = sg.tile([K, 1], f32, tag="bout")
        nc.sync.dma_start(out=s_bout[:], in_=bout[:])
        s_eA = sg.tile([K, K], bf16, tag="eA")
        nc.sync.dma_start(out=s_eA[:], in_=expA[:])
        s_eAT = sg.tile([K, K], bf16, tag="eAT")
        nc.sync.dma_start(out=s_eAT[:], in_=expAT[:])
        s_A = sg.tile([K, K], bf16, tag="A")
        nc.sync.dma_start(out=s_A[:], in_=Abf[:])
        s_ws = sg.tile([1, K], f32, tag="ws")
        nc.sync.dma_start(out=s_ws[:], in_=wstar[:])
        s_t1h = sg.tile([K, T * BL], bf16, tag="t1h")
        nc.sync.dma_start(out=s_t1h[:], in_=t1h[:])
        s_tnx = sg.tile([K, T * BL], bf16, tag="tnx")
        nc.sync.dma_start(out=s_tnx[:], in_=tnx[:])
        s_lnm = sg.tile([1, (NRESC + 1) * 2 * BL], f32, tag="lnm")
        nc.sync.dma_start(out=s_lnm[:], in_=lnmask[:])

        ones = sg.tile([128, K], f32, tag="ones")
        nc.vector.memset(ones[:], 1.0)
        ident = sg.tile([128, 128], bf16, tag="ident")
        make_identity(nc, ident[:])

        # mask replica: (128, T, BL), col = t*16+b, broadcast across partitions
        maskrep = sg.tile([128, T, BL], u8, tag="maskrep")
        nc.sync.dma_start(
            out=maskrep[:],
            in_=bass.AP(tensor=masku.tensor, offset=masku[:].offset,
                        ap=[[0, 128], [BL, T], [1, BL]]),
        )
        invrep = sg.tile([128, T, BL], u8, tag="invrep")
        nc.sync.dma_start(
            out=invrep[:],
            in_=bass.AP(tensor=invmu.tensor, offset=invmu[:].offset,
                        ap=[[0, 128], [BL, T], [1, BL]]),
        )
        neg50 = sg.tile([128, 1], bf16, tag="neg50")
        nc.vector.memset(neg50[:], -50.0)

        # token indices for gathers (128 per column)
        NT128 = T * BL // 128
        idxall = sg.tile([128, NT128], i32, tag="idxall")
        nc.sync.dma_start(out=idxall[:],
                          in_=bass.AP(tensor=toks.tensor, offset=toks[:].offset,
                                      ap=[[1, 128], [128, NT128]]))

        # emissions (f32) and their exp; pre-bias emit with b_out
        emit = sg.tile([K, T, BL], f32, tag="emit")
        emf0 = emit[:].rearrange("k t b -> k (t b)")
        nc.vector.memset(emf0, 0.0)
        nc.vector.tensor_scalar_add(emf0, emf0, s_bout[:, 0:1])
        expE = emit  # exp taken in place after phase 1 (unary reads are done by then)

        # per-direction rotating transposed-input windows and projection windows
        xtp = ctx.enter_context(tc.tile_pool(name="xtp", bufs=2))
        winp = ctx.enter_context(tc.tile_pool(name="winp", bufs=2))

        # LSTM states, layout [128, dir(2), k(2), BL]
        st_h = sg.tile([128, 2, 2, BL], bf16, tag="st_h")
        st_c = sg.tile([128, 2, 2, BL], f32, tag="st_c")
        nc.vector.memset(st_h[:], 0.0)
        nc.vector.memset(st_c[:], 0.0)

        # CRF state S[:, 0]=alpha D, S[:, 1]=beta B; ln-compensation buffer
        S = sg.tile([K, 2, BL], f32, tag="S")
        lnbuf = sg.tile([1, NRESC + 1, 2, BL], f32, tag="lnbuf")
        Uacc = sg.tile([K, BL], f32, tag="Uacc")
        nc.vector.memset(Uacc[:], 0.0)
        TRacc = sg.tile([K, BL], f32, tag="TRacc")
        nc.vector.memset(TRacc[:], 0.0)

        def gather_window(w, d):
            """Gather+transpose window w into a rotating per-direction xT tile."""
            xT = xtp.tile([128, 2, 512], bf16, tag=f"xT{d}", name=f"xT{d}")
            for g in range(4):
                j = w * 4 + g
                xg = gat.tile([128, E], bf16, tag="xg")
                nc.gpsimd.indirect_dma_start(
                    out=xg[:], out_offset=None, in_=emb[:],
                    in_offset=bass.IndirectOffsetOnAxis(ap=idxall[:, j:j + 1], axis=0),
                )
                for k in range(2):
                    pst = ps_t.tile([128, 128], bf16, tag="pst")
                    nc.tensor.transpose(out=pst[:], in_=xg[:, k * 128:(k + 1) * 128],
                                        identity=ident[:])
                    if (g + k) % 2 == 0:
                        nc.vector.tensor_copy(xT[:, k, g * 128:(g + 1) * 128], pst[:])
                    else:
                        nc.scalar.activation(xT[:, k, g * 128:(g + 1) * 128], pst[:], AF.Copy)
            return xT

        def make_window(xT, d, w):
            """Project a gathered window for direction d -> win (128, 8, BL, WIN)
            bf16, bias folded via an extra matmul. psw column order: (b, t).
            For the bwd stream, clamp the i-gate pre-activation to -50 at padded
            positions so the cell state stays exactly 0 through the pad suffix."""
            win = winp.tile([128, 8, WIN, BL], bf16, tag=f"win{d}", name=f"win{d}")
            for m in range(8):
                psw = ps_w.tile([128, 512], f32, tag="psw")
                for k in range(2):
                    nc.tensor.matmul(psw[:], lhsT=s_wih[d][:, k, m * 128:(m + 1) * 128],
                                     rhs=xT[:, k, :], start=(k == 0), stop=False)
                nc.tensor.matmul(psw[:], lhsT=s_bihT[d][:, m, :], rhs=onesb[:],
                                 start=False, stop=True)
                wv = win[:, m].rearrange("p t b -> p (t b)")
                if m % 2 == 0:
                    nc.scalar.activation(wv, psw[:], AF.Copy)
                else:
                    nc.vector.tensor_copy(wv, psw[:])
            if d == "b":
                pred = bass.AP(tensor=invrep.tensor,
                               offset=invrep[0:128, w * WIN, 0:BL].offset,
                               ap=[invrep[:].ap[0], [0, 2], [BL, WIN], [1, BL]])
                data = bass.AP(tensor=neg50.tensor, offset=neg50[:].offset,
                               ap=[neg50[:].ap[0], [0, 2], [0, WIN], [0, BL]])
                nc.vector.copy_predicated(win[:, 0:2], pred, data)
            return win

        # mask AP helpers --------------------------------------------------
        def mask_joint(tlo, thi, parts, reps):
            """(parts, 2, reps, BL) AP over maskrep: dir0 at t=tlo, dir1 at t=thi."""
            base = maskrep[0:parts, tlo, 0:BL]
            return bass.AP(tensor=base.tensor, offset=base.offset,
                           ap=[base.ap[0], [(thi - tlo) * BL, 2], [0, reps], [1, BL]])

        def mask_one(t, parts, reps):
            base = maskrep[0:parts, t, 0:BL]
            if reps == 1:
                return base
            return bass.AP(tensor=base.tensor, offset=base.offset,
                           ap=[base.ap[0], [0, reps], [1, BL]])

        # -------- gold-score chunk work (interleaved into phase 1) --------
        NCH = 16
        CW = T * BL // NCH          # 512 cols per chunk

        def trans_chunk(ci):
            """TRacc += reduce_t(A[tag, :] * shifted-onehot) for chunk ci."""
            psa = ps_w.tile([K, CW], f32, tag="psw", name="psa")
            nc.tensor.matmul(psa[:], lhsT=s_A[:], rhs=s_t1h[:, ci * CW:(ci + 1) * CW],
                             start=True, stop=True)
            um = fin.tile([K, CW], f32, tag="um")
            nc.vector.tensor_tensor(um[:], psa[:], s_tnx[:, ci * CW:(ci + 1) * CW], op=OP.mult)
            ur = fin.tile([K, BL], f32, tag="ur")
            umr = bass.AP(tensor=um.tensor, offset=um[:].offset,
                          ap=[um[:].ap[0], [1, BL], [BL, CW // BL]])
            nc.vector.tensor_reduce(ur[:], umr, axis=mybir.AxisListType.X, op=OP.add)
            nc.vector.tensor_tensor(TRacc[:], TRacc[:], ur[:], op=OP.add)

        def unary_chunk(ci):
            """Uacc += reduce_t(onehot * emit) for chunk ci (emit must be complete)."""
            um = fin.tile([K, CW], f32, tag="um")
            ef = emit[:].rearrange("k t b -> k (t b)")
            nc.gpsimd.tensor_tensor(um[:], s_t1h[:, ci * CW:(ci + 1) * CW],
                                    ef[:, ci * CW:(ci + 1) * CW], op=OP.mult)
            ur = fin.tile([K, BL], f32, tag="ur")
            umr = bass.AP(tensor=um.tensor, offset=um[:].offset,
                          ap=[um[:].ap[0], [1, BL], [BL, CW // BL]])
            nc.vector.tensor_reduce(ur[:], umr, axis=mybir.AxisListType.X, op=OP.add)
            nc.vector.tensor_tensor(Uacc[:], Uacc[:], ur[:], op=OP.add)

        # emit chunk completion wall-step: chunk ci covers t in [ci*32,(ci+1)*32)
        unary_sched = {}
        for ci in range(NCH):
            t0c, t1c = ci * (T // NCH), (ci + 1) * (T // NCH) - 1
            done = max(t1c, T - 1 - t0c) + 1   # +1: emit adds land one step late
            unary_sched.setdefault(min(done, T), []).append(ci)

        # ---------------- phase 1: interleaved fwd/bwd LSTM ----------------
        # prefetch first windows
        win_cur = {"f": make_window(gather_window(0, "f"), "f", 0),
                   "b": make_window(gather_window(15, "b"), "b", 15)}
        win_nxt = {}
        xt_nxt = {}

        warm = ps_s.tile([1, 1], f32, tag="pssm")
        nc.tensor.matmul(warm[:], lhsT=ident[:, 0:1], rhs=ident[:, 0:1], start=True, stop=True)

        def do_pse(s):
            """Emission matmuls + delayed joint emit add for step s."""
            tf, tb = s, T - 1 - s
            tlo, thi = (tf, tb) if tf < tb else (tb, tf)
            jf = 0 if tf < tb else 1
            pse = ps_s.tile([K, 2, BL], f32, tag="pssm", name="pse")
            # single accumulation group: one bank-clear for all four matmuls
            nc.tensor.matmul(pse[:, jf], lhsT=s_wout[:, 0, :], rhs=st_h[:, 0, 0, :],
                             start=True, stop=False)
            nc.tensor.matmul(pse[:, jf], lhsT=s_wout[:, 1, :], rhs=st_h[:, 0, 1, :],
                             start=False, stop=False)
            nc.tensor.matmul(pse[:, 1 - jf], lhsT=s_wout[:, 2, :], rhs=st_h[:, 1, 0, :],
                             start=False, stop=False)
            nc.tensor.matmul(pse[:, 1 - jf], lhsT=s_wout[:, 3, :], rhs=st_h[:, 1, 1, :],
                             start=False, stop=True)
            eap = bass.AP(tensor=emit.tensor, offset=emit[:, tlo, :].offset,
                          ap=[emit[:].ap[0], [(thi - tlo) * BL, 2], [1, BL]])
            nc.vector.tensor_tensor(eap, eap, pse[:], op=OP.add)

        def burst(d, di, s):
            """Recurrence + window-injection matmuls for one stream."""
            toff = s % WIN
            tof = toff if d == "f" else WIN - 1 - toff
            psg = ps_g.tile([128, 8, BL], f32, tag=f"psg{d}", name=f"psg{d}")
            for m in range(8):
                for k in range(2):
                    nc.tensor.matmul(psg[:, m], lhsT=s_whh[d][:, k, m * 128:(m + 1) * 128],
                                     rhs=st_h[:, di, k, :], start=(m == 0 and k == 0), stop=False)
            wslice = bass.AP(tensor=win_cur[d].tensor,
                             offset=win_cur[d][:, :, tof, :].offset,
                             ap=[win_cur[d][:].ap[0], [BL * WIN, 8], [1, BL]])
            nc.tensor.matmul(psg[:], lhsT=ident[:], rhs=wslice, start=False, stop=True)
            return psg

        for s in range(T):
            blk, toff = divmod(s, WIN)

            # ---- stream f: matmuls, activations, cell update (all in place) ----
            psgf = burst("f", 0, s)
            gf = tmp.tile([128, 8, BL], f32, tag="gatf", name="gatf")
            nc.scalar.activation(gf[:], psgf[:], AF.Sigmoid)
            nc.gpsimd.tensor_scalar(gf[:, 6:8, :], gf[:, 6:8, :], 2.0, -1.0,
                                    op0=OP.mult, op1=OP.add)
            igf = tmp.tile([128, 2, BL], f32, tag="igf", name="igf")
            nc.gpsimd.tensor_tensor(igf[:], gf[:, 0:2, :], gf[:, 6:8, :], op=OP.mult)
            nc.vector.tensor_tensor(st_c[:, 0], gf[:, 2:4, :], st_c[:, 0], op=OP.mult)
            nc.vector.tensor_tensor(st_c[:, 0], st_c[:, 0], igf[:], op=OP.add)

            # ---- stream b: matmuls + emissions of step s-1 on PE ----
            psgb = burst("b", 1, s)
            if s > 0:
                do_pse(s - 1)
            gb = tmp.tile([128, 8, BL], f32, tag="gatb", name="gatb")
            nc.scalar.activation(gb[:], psgb[:], AF.Sigmoid)

            # ---- stream f tail: tanh(c), h ----
            thf = tmp.tile([128, 2, BL], f32, tag="thf", name="thf")
            nc.scalar.activation(thf[:], st_c[:, 0], AF.Tanh)
            nc.gpsimd.tensor_scalar(gb[:, 6:8, :], gb[:, 6:8, :], 2.0, -1.0,
                                    op0=OP.mult, op1=OP.add)
            igb = tmp.tile([128, 2, BL], f32, tag="igb", name="igb")
            nc.gpsimd.tensor_tensor(igb[:], gb[:, 0:2, :], gb[:, 6:8, :], op=OP.mult)
            nc.vector.tensor_tensor(st_c[:, 1], gb[:, 2:4, :], st_c[:, 1], op=OP.mult)
            nc.vector.tensor_tensor(st_h[:, 0], gf[:, 4:6, :], thf[:], op=OP.mult)
            nc.vector.tensor_tensor(st_c[:, 1], st_c[:, 1], igb[:], op=OP.add)
            thb = tmp.tile([128, 2, BL], f32, tag="thb", name="thb")
            nc.scalar.activation(thb[:], st_c[:, 1], AF.Tanh)
            nc.vector.tensor_tensor(st_h[:, 1], gb[:, 4:6, :], thb[:], op=OP.mult)

            # window prefetch/projection for the next block, spread over the block
            if blk < NW - 1:
                if toff == 4:
                    xt_nxt["f"] = gather_window(blk + 1, "f")
                if toff == 12:
                    xt_nxt["b"] = gather_window(NW - 2 - blk, "b")
                if toff == 20:
                    win_nxt["f"] = make_window(xt_nxt["f"], "f", blk + 1)
                if toff == 26:
                    win_nxt["b"] = make_window(xt_nxt["b"], "b", NW - 2 - blk)
                if toff == WIN - 1:
                    win_cur = dict(win_nxt)

            # interleave gold-score chunks
            if s % 32 == 16 and s // 32 < NCH // 2:
                ci = s // 32
                trans_chunk(2 * ci)
                trans_chunk(2 * ci + 1)
            for ci in unary_sched.get(s, []):
                unary_chunk(ci)

        do_pse(T - 1)
        for ci in unary_sched.get(T, []):
            unary_chunk(ci)

        # ---------------- phase 1.5: bulk exp ----------------
        ef = emit[:].rearrange("k t b -> k (t b)")
        xf = expE[:].rearrange("k t b -> k (t b)")
        for q in range(4):
            sl = slice(q * T * BL // 4, (q + 1) * T * BL // 4)
            nc.scalar.activation(xf[:, sl], ef[:, sl], AF.Exp)

        # ---------------- phase 2: CRF alpha/beta meeting at TME ----------------
        # Alpha runs in Q-space (Q_t = expA^T @ D_t) so both streams share the
        # "multiply by expE, then matmul" shape:
        #   alpha (dir 0): S0 <- masked_{m[ta]}  (expA^T @ (S0 * expE[ta])),  ta = 0..TME-1
        #   beta  (dir 1): S1 <- masked_{m[te]}  (expA   @ (S1 * expE[te])),  te = 511..TME+1
        # final: logZ = ln sum_i (expE[TME] * Q_{TME-1} * B_TME)[i] + ln-comp terms
        nc.vector.memset(S[:], 1.0)

        nrs = 0
        for kk in range(TME):
            ta = kk                # alpha uses expE[ta], mask[ta]
            te = T - kk            # beta uses expE[te], mask[te]; skip kk=0 (te=512)
            bp = tmp.tile([K, 2, BL], bf16, tag="bp", name="bp")
            psj = ps_s.tile([K, 2, BL], f32, tag="pssm", name="psj")
            if kk == 0:
                nc.vector.tensor_tensor(bp[:, 0], S[:, 0], expE[:, ta, :], op=OP.mult)
                nc.tensor.matmul(psj[:, 0], lhsT=s_eA[:], rhs=bp[:, 0], start=True, stop=True)
                nc.vector.copy_predicated(S[:, 0], mask_one(ta, K, 1), psj[:, 0])
                continue
            # joint: dir0 alpha reads expE[ta], dir1 beta reads expE[te]
            xap = bass.AP(tensor=expE.tensor, offset=expE[:, ta, :].offset,
                          ap=[expE[:].ap[0], [(te - ta) * BL, 2], [1, BL]])
            nc.vector.tensor_tensor(bp[:], S[:], xap, op=OP.mult)
            nc.tensor.matmul(psj[:, 0], lhsT=s_eA[:], rhs=bp[:, 0], start=True, stop=True)
            nc.tensor.matmul(psj[:, 1], lhsT=s_eAT[:], rhs=bp[:, 1], start=True, stop=True)
            nc.vector.copy_predicated(S[:], mask_joint(ta, te, K, 1), psj[:])

            if kk % RESC == 0 and kk >= 8 and kk <= 248:
                # rescale both streams; ln terms batched at the end
                pss = ps_s.tile([1, 2 * BL], f32, tag="pssm", name="pss")
                nc.tensor.matmul(pss[:], lhsT=ones[0:K, 0:1],
                                 rhs=S[:].rearrange("k d b -> k (d b)"), start=True, stop=True)
                nc.vector.tensor_copy(lnbuf[:, nrs].rearrange("o d b -> o (d b)"), pss[:])
                rr = tmp.tile([1, 2 * BL], f32, tag="rr")
                nc.vector.reciprocal(rr[:], pss[:])
                psr = ps_s.tile([K, 2 * BL], f32, tag="pssm", name="psr")
                nc.tensor.matmul(psr[:], lhsT=ones[0:1, 0:K], rhs=rr[:], start=True, stop=True)
                sc = tmp.tile([K, 2, BL], f32, tag="sc")
                nc.vector.tensor_tensor(sc[:].rearrange("k d b -> k (d b)"),
                                        S[:].rearrange("k d b -> k (d b)"), psr[:], op=OP.mult)
                nc.vector.copy_predicated(S[:], mask_joint(ta, te, K, 1), sc[:])
                nrs += 1
        assert nrs == NRESC, nrs

        # final compensated rescale (bounds the final Ln input); plain scaling,
        # the lnmask row gates S1's compensation to columns that use it
        pss = ps_s.tile([1, 2 * BL], f32, tag="pssm", name="pssf")
        nc.tensor.matmul(pss[:], lhsT=ones[0:K, 0:1],
                         rhs=S[:].rearrange("k d b -> k (d b)"), start=True, stop=True)
        nc.vector.tensor_copy(lnbuf[:, NRESC].rearrange("o d b -> o (d b)"), pss[:])
        rr = tmp.tile([1, 2 * BL], f32, tag="rr")
        nc.vector.reciprocal(rr[:], pss[:])
        psr = ps_s.tile([K, 2 * BL], f32, tag="pssm", name="psrf")
        nc.tensor.matmul(psr[:], lhsT=ones[0:1, 0:K], rhs=rr[:], start=True, stop=True)
        nc.vector.tensor_tensor(S[:].rearrange("k d b -> k (d b)"),
                                S[:].rearrange("k d b -> k (d b)"), psr[:], op=OP.mult)

        # ---------------- finalize ----------------
        # logZ = ln(sum_i D[i]*B[i]) + sum(masked ln rescale terms)
        lns = fin.tile([1, (NRESC + 1) * 2 * BL], f32, tag="lns")
        nc.scalar.activation(lns[:], lnbuf[:].rearrange("o r d b -> o (r d b)"), AF.Ln)
        nc.vector.tensor_tensor(lns[:], lns[:], s_lnm[:], op=OP.mult)
        lnred = fin.tile([1, BL], f32, tag="lnred")
        lnsr = bass.AP(tensor=lns.tensor, offset=lns[:].offset,
                       ap=[lns[:].ap[0], [1, BL], [BL, (NRESC + 1) * 2]])
        nc.vector.tensor_reduce(lnred[:], lnsr, axis=mybir.AxisListType.X, op=OP.add)

        # combine vector V: active cols (len > TME) use expE[TME]*beta,
        # frozen cols (len <= TME) use w* (recovers sum_i D_i from Q-space)
        psR = ps_s.tile([K, BL], f32, tag="pssm", name="psR")
        nc.tensor.matmul(psR[:], lhsT=s_ws[:], rhs=ones[0:1, 0:BL], start=True, stop=True)
        V = fin.tile([K, BL], f32, tag="V")
        nc.vector.tensor_copy(V[:], psR[:])
        EV = fin.tile([K, BL], f32, tag="EV")
        nc.vector.tensor_tensor(EV[:], expE[:, TME, :], S[:, 1], op=OP.mult)
        nc.vector.copy_predicated(V[:], mask_one(TME, K, 1), EV[:])
        zt = fin.tile([K, BL], f32, tag="zt")
        nc.vector.tensor_tensor(zt[:], S[:, 0], V[:], op=OP.mult)
        psz = ps_s.tile([1, BL], f32, tag="pssm", name="psz")
        nc.tensor.matmul(psz[:], lhsT=ones[0:K, 0:1], rhs=zt[:], start=True, stop=True)
        logZ = fin.tile([1, BL], f32, tag="logZ")
        nc.scalar.activation(logZ[:], psz[:], AF.Ln)
        nc.vector.tensor_tensor(logZ[:], logZ[:], lnred[:], op=OP.add)

        # gold score = colsum(Uacc) + colsum(TRacc)
        nc.vector.tensor_tensor(Uacc[:], Uacc[:], TRacc[:], op=OP.add)
        psu = ps_s.tile([1, BL], f32, tag="pssm", name="psu")
        nc.tensor.matmul(psu[:], lhsT=ones[0:K, 0:1], rhs=Uacc[:], start=True, stop=True)

        res = fin.tile([1, BL], f32, tag="res")
        nc.vector.tensor_tensor(res[:], logZ[:], psu[:], op=OP.subtract)
        nc.sync.dma_start(out=out_loss[:], in_=res[:])

    nc.compile()
    return nc, names


def _prep_core(inputs, kcore):
    """Per-core host-side input prep (index plumbing + layout shuffles)."""
    import ml_dtypes
    bf = ml_dtypes.bfloat16
    s = slice(kcore * BL, (kcore + 1) * BL)
    sent = np.asarray(inputs["sentences"][s])          # (16, 512) i32
    tags = np.asarray(inputs["tags"][s])               # (16, 512) i32
    mask = (sent != PAD_IDX)                           # (16, 512)

    # gate-row permutation [i,f,g,o] -> [i,f,o,g]; g rows scaled x2 so that
    # tanh(g) can be computed as 2*sigmoid(2g)-1 with a single Sigmoid op
    perm = np.concatenate([np.arange(0, 2 * H), np.arange(3 * H, 4 * H),
                           np.arange(2 * H, 3 * H)])
    gsc = np.ones((4 * H, 1), np.float32)
    gsc[3 * H:] = 2.0

    toks = sent.reshape(BL, NW, WIN).transpose(1, 2, 0).reshape(T * BL, 1)
    oh = (tags[:, :, None] == np.arange(K)[None, None, :])
    t1h = (oh & mask[:, :, None]).transpose(2, 1, 0).reshape(K, T * BL)
    tnxm = np.zeros((BL, T, K), np.float32)
    tnxm[:, :-1, :] = (oh[:, 1:, :] & mask[:, 1:, None]).astype(np.float32)
    tnx = tnxm.transpose(2, 1, 0).reshape(K, T * BL)

    # lnmask: mask value at the rescale checkpoints, layout (r, dir, b)
    lnm = np.zeros((NRESC + 1, 2, BL), np.float32)
    for r in range(NRESC):
        kk = 8 * (r + 1)
        lnm[r, 0] = mask[:, kk].astype(np.float32)       # alpha gate: mask[ta=kk]
        lnm[r, 1] = mask[:, T - kk].astype(np.float32)   # beta gate: mask[te=512-kk]
    lnm[NRESC, 0] = 1.0                                  # final rescale: S0 always
    lnm[NRESC, 1] = mask[:, TME].astype(np.float32)      # S1 only if used

    A = np.asarray(inputs["transition"], np.float32)

    def wperm(w):
        return np.ascontiguousarray((np.asarray(w)[perm] * gsc).T).astype(bf)

    def bperm(b):
        return np.asarray(b)[perm] * gsc[:, 0]

    m = {
        "toks": toks.astype(np.int32),
        "masku": mask.T.astype(np.uint8).reshape(1, T * BL),
        "invmu": (~mask).T.astype(np.uint8).reshape(1, T * BL),
        "t1h": t1h.astype(bf),
        "tnx": tnx.astype(bf),
        "lnmask": lnm.reshape(1, (NRESC + 1) * 2 * BL),
        "emb": np.asarray(inputs["embedding"]).astype(bf),
        "wih_f": wperm(inputs["w_ih_f"]),
        "wih_b": wperm(inputs["w_ih_b"]),
        "whh_f": wperm(inputs["w_hh_f"]),
        "whh_b": wperm(inputs["w_hh_b"]),
        "bih_f": np.ascontiguousarray(bperm(inputs["b_f"]).reshape(8, 128).T).astype(np.float32),
        "bih_b": np.ascontiguousarray(bperm(inputs["b_b"]).reshape(8, 128).T).astype(np.float32),
        "bihT_f": bperm(inputs["b_f"]).reshape(1, 8 * 128).astype(bf),
        "bihT_b": bperm(inputs["b_b"]).reshape(1, 8 * 128).astype(bf),
        "woutT": np.ascontiguousarray(np.asarray(inputs["w_out"]).T.reshape(4, 128, K)).astype(bf),
        "bout": np.asarray(inputs["b_out"]).reshape(K, 1).astype(np.float32),
        "expA": np.exp(A).astype(bf),
        "expAT": np.ascontiguousarray(np.exp(A).T).astype(bf),
        "Abf": A.astype(bf),
        "wstar": np.linalg.solve(np.exp(A), np.ones(K)).reshape(1, K).astype(np.float32),
    }
    return m


def kernel(**inputs):
    from concourse.bass_utils import run_bass_kernel_spmd

    if "prog" not in _cache:
        _cache["prog"] = _build_program()
    nc, names = _cache["prog"]

    in_maps = []
    for kcore in range(NCORES):
        m = _prep_core(inputs, kcore)
        in_maps.append({names[kk]: vv for kk, vv in m.items()})

    res = run_bass_kernel_spmd(nc, in_maps, core_ids=list(range(NCORES)),
                               **_cache.get("run_kwargs", {}))
    out = np.concatenate([r[names["out"]].reshape(BL) for r in res.results])
    _cache["last_results"] = res
    return out.astype(np.float32)
